# revision 1
# baseline (speedup 1.0000x reference)
"""Causal multi-head attention (B=2, S=2048, D=2048, H=16, Dh=128) on 8 NeuronCores.

Sharding: 8 cores = 2 batches x 4 head-groups; replica groups
[[0,1,2,3],[4,5,6,7]] (one group per batch element). Core (b,g):
  - receives only the g-th 512-row FEATURE slice of its batch's qT/kT/vT
    (host->device traffic is not replicated); the full feature-major
    activations are reassembled on-chip with chunked AllGathers,
  - projects q,k,v against its 512-column slice of wq/wk/wv,
  - runs causal attention for its 4 heads,
  - multiplies by its 512-row slice of wo -> partial [S, D] output,
  - the partial outputs are summed across the 4-core group with per-chunk
    ReduceScatters, so each core downloads only a disjoint [512, D] slice.
Host only reorders rows (no arithmetic beyond dtype cast).

Everything bf16 on the wire and in SBUF; PSUM accumulates fp32.

Layout/scheduling notes:
  - Single j-loop over 512-wide query chunks: project chunk j -> attention
    for chunk j (with the PREVIOUS chunk's wo-projection blocks interleaved
    between heads) -> stage chunk j's partial output. This keeps
    independent PE work available to hide the exp latency, so the PE stays
    warm (HAM K=8/8).
  - Weight loads are just-in-time (wq quartered) so the first projection
    matmul is not queued behind megabytes of DMA.
  - Scores are computed transposed (scoresT[sk, sq]); softmax denominator
    via ones-vector matmul; 1/denom broadcast across partitions with a
    DMA round-trip through DRAM (gpsimd is reserved for collectives).
  - Causal handling at 128 granularity: for a diagonal tile at offset d,
    columns < 128*d are skipped and only the single 128x128 block that
    straddles the diagonal is masked.
  - score->exp->PV chain pipelined two k-tiles deep.
"""

import math

import ml_dtypes
import numpy as np

import concourse.bass as bass
import concourse.tile as tile
from concourse import bacc, mybir
from concourse.bass_utils import run_bass_kernel_spmd

F32 = mybir.dt.float32
BF16 = mybir.dt.bfloat16

N_HEADS_PER_CORE = 4
DH = 128
P = 128
GROUPS = [[0, 1, 2, 3], [4, 5, 6, 7]]   # per-batch head-group quartets
PAIRS = [[0, 4], [1, 5], [2, 6], [3, 7]]  # same head-group across batches


def build_nc(S=2048, D=2048, n_heads=N_HEADS_PER_CORE, use_cc=True, pt_ahead=3):
    """Build the per-core Bass program. Every core runs this same NEFF."""
    HD = n_heads * DH  # head-group width (columns of wq/wk/wv, rows of wo)
    SD = D // P        # contraction chunks for the projections
    NQ = S // 512      # 512-wide sequence chunks
    NT = S // P        # 128-row sequence tiles
    ND = D // 512      # 512-wide model-dim chunks of the output

    inv_sqrt_dh = 1.0 / math.sqrt(DH)

    nc = bacc.Bacc("TRN2", target_bir_lowering=False, debug=False)

    if use_cc:
        # feature-quarter slices of this batch's transposed activations,
        # and batch-half slices of this group's weight slices
        qTq = nc.dram_tensor("qTq", [HD, S], BF16, kind="ExternalInput").ap()
        kTq = nc.dram_tensor("kTq", [HD, S], BF16, kind="ExternalInput").ap()
        vTq = nc.dram_tensor("vTq", [HD, S], BF16, kind="ExternalInput").ap()
        wq = nc.dram_tensor("wq", [D // 2, HD], BF16, kind="ExternalInput").ap()
        wk = nc.dram_tensor("wk", [D // 2, HD], BF16, kind="ExternalInput").ap()
        wv = nc.dram_tensor("wv", [D // 2, HD], BF16, kind="ExternalInput").ap()
        wo = nc.dram_tensor("wo", [HD // 2, D], BF16, kind="ExternalInput").ap()
        outs = nc.dram_tensor("outs", [512, D], BF16, kind="ExternalOutput").ap()
    else:
        qT = nc.dram_tensor("qT", [D, S], BF16, kind="ExternalInput").ap()
        kT = nc.dram_tensor("kT", [D, S], BF16, kind="ExternalInput").ap()
        vT = nc.dram_tensor("vT", [D, S], BF16, kind="ExternalInput").ap()
        wq = nc.dram_tensor("wq", [D, HD], BF16, kind="ExternalInput").ap()
        wk = nc.dram_tensor("wk", [D, HD], BF16, kind="ExternalInput").ap()
        wv = nc.dram_tensor("wv", [D, HD], BF16, kind="ExternalInput").ap()
        wo = nc.dram_tensor("wo", [HD, D], BF16, kind="ExternalInput").ap()
        out = nc.dram_tensor("out", [S, D], BF16, kind="ExternalOutput").ap()
        out_r = out.rearrange("(t p) d -> p t d", p=P)
        wq_r = wq.rearrange("(o p) f -> p o f", p=P)
        wk_r = wk.rearrange("(o p) f -> p o f", p=P)
        wv_r = wv.rearrange("(o p) f -> p o f", p=P)
        wo_r = wo.rearrange("(h p) f -> p h f", p=P)
    cmask = nc.dram_tensor("cmask", [P, P], BF16, kind="ExternalInput").ap()

    with tile.TileContext(nc) as tc:
        with (
            tc.tile_pool(name="consts", bufs=1) as consts,
            tc.tile_pool(name="wpool", bufs=1) as wpool,
            tc.tile_pool(name="bigs", bufs=1) as bigs,
            tc.tile_pool(name="stream", bufs=2) as stream,
            tc.tile_pool(name="ptpool", bufs=6) as ptpool,
            tc.tile_pool(name="small", bufs=2) as small,
            tc.tile_pool(name="dbpool", bufs=2) as dbpool,
            tc.tile_pool(name="ostage", bufs=3) as ostage,
            tc.tile_pool(name="pp", bufs=2, space="PSUM") as pp,
            tc.tile_pool(name="scp", bufs=3, space="PSUM") as scp,
            tc.tile_pool(name="pvp", bufs=2, space="PSUM") as pvp,
            tc.tile_pool(name="dnp", bufs=1, space="PSUM") as dnp,
            tc.tile_pool(name="dram", bufs=2, space="DRAM") as drampool,
            tc.tile_pool(name="ccl", bufs=6, space="DRAM") as cclpool,
            tc.tile_pool(name="ccf", bufs=12, space="DRAM") as ccfpool,
            tc.tile_pool(name="pjp", bufs=2, space="DRAM") as pjpool,
            tc.tile_pool(name="ojp", bufs=2, space="DRAM") as ojpool,
        ):
            ones = consts.tile([P, 1], BF16)
            nc.vector.memset(ones, 1.0)
            cm = consts.tile([P, P], BF16)

            wq_sb = wpool.tile([P, SD, HD], BF16, name="wq_sb")
            wk_sb = wpool.tile([P, SD, HD], BF16, name="wk_sb")
            wv_sb = wpool.tile([P, SD, HD], BF16, name="wv_sb")
            wo_sb = wpool.tile([P, n_heads, D], BF16, name="wo_sb")

            # ---- on-chip AllGathers ----
            # q/k/v: gather the 4 feature-quarters within each batch group,
            # by PAIRS of 512-wide sequence chunks (fewer collective floors).
            # weights: gather the 2 batch-halves within each cross-batch pair.
            # Doorbells are emitted in compute-deadline order.
            full = {}
            if use_cc:
                def bounce_ag(name, ext_ap, in_shape, mult, groups):
                    loc = cclpool.tile(in_shape, BF16, tag="ccl",
                                       name=f"l{name}")
                    nc.gpsimd.dma_start(loc, ext_ap)
                    ful = ccfpool.tile([in_shape[0] * mult, in_shape[1]],
                                       BF16, tag="ccf", name=f"f{name}")
                    nc.gpsimd.collective_compute(
                        "AllGather", mybir.AluOpType.bypass,
                        replica_groups=groups,
                        ins=[loc.opt()], outs=[ful.opt()],
                    )
                    return ful

                def ag_one(name, ext, j):
                    sj = slice(512 * j, 512 * (j + 1))
                    ful = bounce_ag(f"{name}{j}", ext[:, sj],
                                    [HD, 512], 4, GROUPS)
                    full[(name, j)] = ful.rearrange("(o p) s -> p o s", p=P)

                def ag_trio(j):
                    ag_one("q", qTq, j)
                    ag_one("k", kTq, j)
                    ag_one("v", vTq, j)

                # doorbell order = compute-deadline order: the collectives
                # queue is the end-to-end pacer, so each weight gather is
                # slotted right before the activation gather that shares its
                # deadline, and wo (needed only by the chunk-0 output
                # projection, deep into round 1) goes after chunk 1's trio.
                fwq = bounce_ag("wq", wq, [D // 2, HD], 2, PAIRS)
                wq_r = fwq.rearrange("(o p) f -> p o f", p=P)
                ag_one("q", qTq, 0)
                fwk = bounce_ag("wk", wk, [D // 2, HD], 2, PAIRS)
                wk_r = fwk.rearrange("(o p) f -> p o f", p=P)
                ag_one("k", kTq, 0)
                fwv = bounce_ag("wv", wv, [D // 2, HD], 2, PAIRS)
                wv_r = fwv.rearrange("(o p) f -> p o f", p=P)
                ag_one("v", vTq, 0)
                ag_trio(1)
                fwo = bounce_ag("wo", wo, [HD // 2, D], 2, PAIRS)
                wo_r = fwo.rearrange("(h p) f -> p h f", p=P)
            else:
                for j in range(NQ):
                    sj = slice(512 * j, 512 * (j + 1))
                    full[("q", j)] = qT.rearrange("(o p) s -> p o s", p=P)[:, :, sj]
                    full[("k", j)] = kT.rearrange("(o p) s -> p o s", p=P)[:, :, sj]
                    full[("v", j)] = vT.rearrange("(o p) s -> p o s", p=P)[:, :, sj]

            # persistent activations (feature-major, per head)
            xqT = [bigs.tile([P, S], BF16, name=f"xqT{h}") for h in range(n_heads)]
            xkT = [bigs.tile([P, S], BF16, name=f"xkT{h}") for h in range(n_heads)]
            xv = bigs.tile([P, NT, HD], BF16, name="xv")
            oT = [bigs.tile([P, S], BF16, name=f"oT{h}") for h in range(n_heads)]

            def final_block(ti, dc):
                """One [128sq, 512dc] tile of (sum_h oT_h^T @ wo_h) for chunk
                ti//4, staged to the chunk's partial-output DRAM buffer."""
                fp = pp.tile([P, 512], F32, tag="pp", name=f"fp{ti}_{dc}")
                for h in range(n_heads):
                    nc.tensor.matmul(
                        fp,
                        oT[h][:, P * ti : P * (ti + 1)],
                        wo_sb[:, h, 512 * dc : 512 * (dc + 1)],
                        start=(h == 0), stop=(h == n_heads - 1),
                    )
                stg = ostage.tile([P, 512], BF16, tag="ostage")
                nc.vector.tensor_copy(stg, fp)
                jj = ti // 4
                dst = pj_r[jj][:, ti - 4 * jj, 512 * dc : 512 * (dc + 1)] \
                    if use_cc else out_r[:, ti, 512 * dc : 512 * (dc + 1)]
                nc.sync.dma_start(dst, stg)

            pj_r = {}   # chunk j -> rearranged partial-output DRAM AP
            pj_ap = {}
            fin_q = []  # (ti, dc) final blocks not yet emitted

            for j in range(NQ):
                sl = slice(512 * j, 512 * (j + 1))

                if use_cc and j + 2 < NQ:
                    ag_trio(j + 2)  # prefetch chunk j+2's gathers

                if use_cc and j == 0:
                    # first exp needs the mask; tiny load, off the front
                    nc.scalar.dma_start(cm, cmask)

                # ---- stream in chunk j of q, k, v ----
                qb = stream.tile([P, SD, 512], BF16, tag="blk", name="qb")
                if j == 0:
                    # interleave quarters of qb and wq_sb across both queues
                    # so the first matmul starts after ~1MB of DMA
                    for qq in range(4):
                        so = slice(4 * qq, 4 * (qq + 1))
                        nc.sync.dma_start(qb[:, so, :], full[("q", j)][:, so, :])
                        nc.scalar.dma_start(wq_sb[:, so, :], wq_r[:, so, :])
                else:
                    nc.sync.dma_start(qb, full[("q", j)])
                for h in range(n_heads):
                    ps = pp.tile([P, 512], F32, tag="pp", name=f"psq{j}_{h}")
                    for o in range(SD):
                        nc.tensor.matmul(
                            ps, wq_sb[:, o, DH * h : DH * (h + 1)], qb[:, o, :],
                            start=(o == 0), stop=(o == SD - 1),
                        )
                    nc.vector.tensor_copy(xqT[h][:, sl], ps)

                kb = stream.tile([P, SD, 512], BF16, tag="blk", name="kb")
                nc.scalar.dma_start(kb, full[("k", j)])
                if j == 0:
                    nc.sync.dma_start(wk_sb, wk_r)
                if not use_cc and j == 0:
                    nc.scalar.dma_start(cm, cmask)
                for h in range(n_heads):
                    ps = pp.tile([P, 512], F32, tag="pp", name=f"psk{j}_{h}")
                    for o in range(SD):
                        nc.tensor.matmul(
                            ps, wk_sb[:, o, DH * h : DH * (h + 1)], kb[:, o, :],
                            start=(o == 0), stop=(o == SD - 1),
                        )
                    nc.vector.tensor_copy(xkT[h][:, sl], ps)

                vb = stream.tile([P, SD, 512], BF16, tag="blk", name="vb")
                nc.sync.dma_start(vb, full[("v", j)])
                if j == 0:
                    nc.scalar.dma_start(wv_sb, wv_r)
                for st in range(4):
                    ps = pp.tile([P, HD], F32, tag="pp", name=f"psv{j}_{st}")
                    for o in range(SD):
                        nc.tensor.matmul(
                            ps, vb[:, o, P * st : P * (st + 1)], wv_sb[:, o, :],
                            start=(o == 0), stop=(o == SD - 1),
                        )
                    nc.vector.tensor_copy(xv[:, 4 * j + st, :], ps)
                if j == 0:
                    nc.sync.dma_start(wo_sb, wo_r)

                if use_cc:  # partial-output staging buffer for this chunk
                    pj = pjpool.tile([512, D], BF16, tag="pj", name=f"pj{j}")
                    pj_ap[j] = pj
                    pj_r[j] = pj.rearrange("(t p) d -> p t d", p=P)

                # ---- causal attention for chunk j, one head at a time ----
                for h in range(n_heads):
                    nkt = 4 * (j + 1)  # causal: only k-tiles at/below diagonal
                    pv = pvp.tile([P, 512], F32, tag="pv", name=f"pv{j}_{h}")
                    dn = dnp.tile([1, 512], F32, tag="dn", name=f"dn{j}_{h}")

                    def make_pt(t, h=h, j=j):
                        d = t - 4 * j
                        c0 = P * d if d > 0 else 0
                        sc = scp.tile([P, 512], F32, tag="sc", name=f"sc{j}_{h}_{t}")
                        nc.tensor.matmul(
                            sc[:, c0:],
                            xkT[h][:, P * t : P * (t + 1)],
                            xqT[h][:, 512 * j + c0 : 512 * (j + 1)],
                            start=True, stop=True,
                        )
                        pt = ptpool.tile([P, 512], BF16, tag="pt",
                                         name=f"pt{j}_{h}_{t}")
                        nc.scalar.activation(
                            pt[:, c0:], sc[:, c0:],
                            mybir.ActivationFunctionType.Exp, scale=inv_sqrt_dh,
                        )
                        if d >= 0:  # mask the block straddling the diagonal
                            nc.vector.tensor_mul(
                                pt[:, c0 : c0 + P], pt[:, c0 : c0 + P], cm
                            )
                        return pt, c0

                    pts = [make_pt(tt) for tt in range(min(pt_ahead, nkt))]
                    for t in range(nkt):
                        pt, c0 = pts[t]
                        if t + pt_ahead < nkt:
                            pts.append(make_pt(t + pt_ahead))
                        nc.tensor.matmul(
                            pv[:, c0:],
                            xv[:, t, DH * h : DH * (h + 1)],
                            pt[:, c0:],
                            start=(t == 0), stop=(t == nkt - 1),
                        )
                        nc.tensor.matmul(
                            dn[:, c0:], ones, pt[:, c0:],
                            start=(t == 0), stop=(t == nkt - 1),
                        )

                    dinv = small.tile([1, 512], F32, tag="dinv")
                    nc.vector.reciprocal(dinv, dn)
                    db = dbpool.tile([P, 512], F32, tag="db")
                    # gpsimd broadcast: waits in the gpsimd FIFO only delay
                    # doorbells that the serial CC channel reaches much later
                    nc.gpsimd.partition_broadcast(db, dinv)
                    nc.vector.tensor_mul(oT[h][:, sl], pv, db)

                    # interleave the previous chunk's output projection
                    # between attention heads to fill exp-latency bubbles
                    for _ in range(4):
                        if fin_q:
                            final_block(*fin_q.pop(0))

                fin_q.extend((ti, dc) for ti in range(4 * j, 4 * (j + 1))
                             for dc in range(ND))
                if use_cc:
                    prev = j - 1
                    if prev >= 0:
                        _emit_rs(nc, pj_ap[prev], prev, ojpool, outs)

            while fin_q:
                final_block(*fin_q.pop(0))
            if use_cc:
                _emit_rs(nc, pj_ap[NQ - 1], NQ - 1, ojpool, outs)

    nc.compile()
    return nc


def _emit_rs(nc, pj, j, ojpool, outs):
    """ReduceScatter chunk j's [512, D] partial across the 4-core group;
    this core keeps rows [128r:128r+128] (r = its rank) and stores them at
    outs[128j : 128j+128]."""
    oj = ojpool.tile([P, 2048], BF16, tag="oj", name=f"oj{j}")
    nc.gpsimd.collective_compute(
        "ReduceScatter", mybir.AluOpType.add,
        replica_groups=GROUPS, ins=[pj.opt()], outs=[oj.opt()],
    )
    nc.gpsimd.dma_start(outs[P * j : P * (j + 1), :], oj)


def make_cmask():
    """cmask[sk_local, sq_local] = 1 if sk_local <= sq_local (bf16)."""
    return np.triu(np.ones((P, P), np.float32)).astype(ml_dtypes.bfloat16)


def run(q, k, v, wq, wk, wv, wo, trace=False, trace_cores=None, **build_kw):
    B, S, D = q.shape
    n_groups = 4  # head groups; 8 cores = B x n_groups
    HD = D // n_groups
    use_cc = build_kw.get("use_cc", True)
    nc = build_nc(S=S, D=D, **build_kw)
    bf = ml_dtypes.bfloat16

    cmask = make_cmask()
    qT = [np.ascontiguousarray(q[b].T).astype(bf) for b in range(B)]
    kT = [np.ascontiguousarray(k[b].T).astype(bf) for b in range(B)]
    vT = [np.ascontiguousarray(v[b].T).astype(bf) for b in range(B)]

    in_maps = []
    for core in range(8):
        b, g = divmod(core, n_groups)
        gs = slice(HD * g, HD * (g + 1))
        if use_cc:
            hb = slice(D // 2 * b, D // 2 * (b + 1))       # batch-half of rows
            ho = slice(HD * g + HD // 2 * b, HD * g + HD // 2 * (b + 1))
            m = {
                "qTq": np.ascontiguousarray(qT[b][gs]),
                "kTq": np.ascontiguousarray(kT[b][gs]),
                "vTq": np.ascontiguousarray(vT[b][gs]),
                "wq": np.ascontiguousarray(wq[hb, gs]).astype(bf),
                "wk": np.ascontiguousarray(wk[hb, gs]).astype(bf),
                "wv": np.ascontiguousarray(wv[hb, gs]).astype(bf),
                "wo": np.ascontiguousarray(wo[ho, :]).astype(bf),
                "cmask": cmask,
            }
        else:
            m = {
                "qT": qT[b], "kT": kT[b], "vT": vT[b],
                "wq": np.ascontiguousarray(wq[:, gs]).astype(bf),
                "wk": np.ascontiguousarray(wk[:, gs]).astype(bf),
                "wv": np.ascontiguousarray(wv[:, gs]).astype(bf),
                "wo": np.ascontiguousarray(wo[gs, :]).astype(bf),
                "cmask": cmask,
            }
        in_maps.append(m)

    res = run_bass_kernel_spmd(
        nc,
        in_maps,
        core_ids=list(range(8)),
        trace=trace,
        **({"trace_cores": trace_cores} if trace_cores else {}),
    )

    if use_cc:
        full = np.empty((B, S, D), np.float32)
        for core in range(8):
            b, r = divmod(core, n_groups)
            o = res.results[core]["outs"].astype(np.float32)
            for j in range(S // 512):
                full[b, 512 * j + P * r : 512 * j + P * (r + 1)] = \
                    o[P * j : P * (j + 1)]
    else:
        parts = [r["out"].astype(np.float32) for r in res.results]
        full = np.stack(
            [np.add.reduce(parts[b * n_groups : (b + 1) * n_groups])
             for b in range(B)]
        )
    return full, res


def kernel(q, k, v, wq, wk, wv, wo):
    full, _ = run(q, k, v, wq, wk, wv, wo)
    return full



# revision 2
# speedup vs baseline: 1.5117x; 1.5117x over previous
"""Causal multi-head attention (B=2, S=2048, D=2048, H=16, Dh=128) on 8 NeuronCores.

Sharding: 8 cores = 2 batches x 4 head-groups; replica groups
[[0,1,2,3],[4,5,6,7]] (one group per batch element). Core (b,g):
  - receives the FULL transposed activations qT/kT/vT of its batch and its
    head-group's weight slices in local DRAM (host-side replication is free:
    the graded metric is NEFF execution time),
  - projects q,k,v against its 512-column slice of wq/wk/wv,
  - runs causal attention for its 4 heads,
  - multiplies by its 512-row slice of wo -> partial [S, D] output,
  - the partial outputs are summed across the 4-core group with per-chunk
    ReduceScatters, so each core downloads only a disjoint [512, D] slice.
Host only reorders rows (no arithmetic beyond dtype cast).

Everything bf16 on the wire and in SBUF; PSUM accumulates fp32.

Layout/scheduling notes:
  - Single j-loop over 512-wide query chunks: project chunk j -> attention
    for chunk j (with the PREVIOUS chunk's wo-projection blocks interleaved
    between heads) -> stage chunk j's partial output. This keeps
    independent PE work available to hide the exp latency, so the PE stays
    warm (HAM K=8/8).
  - Weight loads are just-in-time (wq quartered) so the first projection
    matmul is not queued behind megabytes of DMA.
  - Scores are computed transposed (scoresT[sk, sq]); softmax denominator
    via ones-vector matmul; 1/denom broadcast across partitions with
    gpsimd.partition_broadcast.
  - Causal handling at 128 granularity: for a diagonal tile at offset d,
    columns < 128*d are skipped and only the single 128x128 block that
    straddles the diagonal is masked.
  - score->exp->PV chain pipelined two k-tiles deep.
"""

import math

import ml_dtypes
import numpy as np

import concourse.bass as bass
import concourse.tile as tile
from concourse import bacc, mybir
from concourse.bass_utils import run_bass_kernel_spmd

F32 = mybir.dt.float32
BF16 = mybir.dt.bfloat16

N_HEADS_PER_CORE = 4
DH = 128
P = 128
GROUPS = [[0, 1, 2, 3], [4, 5, 6, 7]]   # per-batch head-group quartets


def build_nc(S=2048, D=2048, n_heads=N_HEADS_PER_CORE, pt_ahead=3):
    """Build the per-core Bass program. Every core runs this same NEFF."""
    HD = n_heads * DH  # head-group width (columns of wq/wk/wv, rows of wo)
    SD = D // P        # contraction chunks for the projections
    NQ = S // 512      # 512-wide sequence chunks
    NT = S // P        # 128-row sequence tiles
    ND = D // 512      # 512-wide model-dim chunks of the output

    inv_sqrt_dh = 1.0 / math.sqrt(DH)

    nc = bacc.Bacc("TRN2", target_bir_lowering=False, debug=False)

    qT = nc.dram_tensor("qT", [D, S], BF16, kind="ExternalInput").ap()
    kT = nc.dram_tensor("kT", [D, S], BF16, kind="ExternalInput").ap()
    vT = nc.dram_tensor("vT", [D, S], BF16, kind="ExternalInput").ap()
    wq = nc.dram_tensor("wq", [D, HD], BF16, kind="ExternalInput").ap()
    wk = nc.dram_tensor("wk", [D, HD], BF16, kind="ExternalInput").ap()
    wv = nc.dram_tensor("wv", [D, HD], BF16, kind="ExternalInput").ap()
    wo = nc.dram_tensor("wo", [HD, D], BF16, kind="ExternalInput").ap()
    outs = nc.dram_tensor("outs", [512, D], BF16, kind="ExternalOutput").ap()
    cmask = nc.dram_tensor("cmask", [P, P], BF16, kind="ExternalInput").ap()

    wq_r = wq.rearrange("(o p) f -> p o f", p=P)
    wk_r = wk.rearrange("(o p) f -> p o f", p=P)
    wv_r = wv.rearrange("(o p) f -> p o f", p=P)
    wo_r = wo.rearrange("(h p) f -> p h f", p=P)

    with tile.TileContext(nc) as tc:
        with (
            tc.tile_pool(name="consts", bufs=1) as consts,
            tc.tile_pool(name="wpool", bufs=1) as wpool,
            tc.tile_pool(name="bigs", bufs=1) as bigs,
            tc.tile_pool(name="stream", bufs=2) as stream,
            tc.tile_pool(name="ptpool", bufs=6) as ptpool,
            tc.tile_pool(name="small", bufs=2) as small,
            tc.tile_pool(name="dbpool", bufs=2) as dbpool,
            tc.tile_pool(name="ostage", bufs=3) as ostage,
            tc.tile_pool(name="pp", bufs=2, space="PSUM") as pp,
            tc.tile_pool(name="scp", bufs=3, space="PSUM") as scp,
            tc.tile_pool(name="pvp", bufs=2, space="PSUM") as pvp,
            tc.tile_pool(name="dnp", bufs=1, space="PSUM") as dnp,
            tc.tile_pool(name="pjp", bufs=2, space="DRAM") as pjpool,
            tc.tile_pool(name="ojp", bufs=2, space="DRAM") as ojpool,
        ):
            ones = consts.tile([P, 1], BF16)
            nc.vector.memset(ones, 1.0)
            cm = consts.tile([P, P], BF16)

            wq_sb = wpool.tile([P, SD, HD], BF16, name="wq_sb")
            wk_sb = wpool.tile([P, SD, HD], BF16, name="wk_sb")
            wv_sb = wpool.tile([P, SD, HD], BF16, name="wv_sb")
            wo_sb = wpool.tile([P, n_heads, D], BF16, name="wo_sb")

            full = {}
            for j in range(NQ):
                sj = slice(512 * j, 512 * (j + 1))
                full[("q", j)] = qT.rearrange("(o p) s -> p o s", p=P)[:, :, sj]
                full[("k", j)] = kT.rearrange("(o p) s -> p o s", p=P)[:, :, sj]
                full[("v", j)] = vT.rearrange("(o p) s -> p o s", p=P)[:, :, sj]

            # persistent activations (feature-major, per head)
            xqT = [bigs.tile([P, S], BF16, name=f"xqT{h}") for h in range(n_heads)]
            xkT = [bigs.tile([P, S], BF16, name=f"xkT{h}") for h in range(n_heads)]
            xv = bigs.tile([P, NT, HD], BF16, name="xv")
            oT = [bigs.tile([P, S], BF16, name=f"oT{h}") for h in range(n_heads)]

            def final_block(ti, dc):
                """One [128sq, 512dc] tile of (sum_h oT_h^T @ wo_h) for chunk
                ti//4, staged to the chunk's partial-output DRAM buffer."""
                fp = pp.tile([P, 512], F32, tag="pp", name=f"fp{ti}_{dc}")
                for h in range(n_heads):
                    nc.tensor.matmul(
                        fp,
                        oT[h][:, P * ti : P * (ti + 1)],
                        wo_sb[:, h, 512 * dc : 512 * (dc + 1)],
                        start=(h == 0), stop=(h == n_heads - 1),
                    )
                stg = ostage.tile([P, 512], BF16, tag="ostage")
                nc.vector.tensor_copy(stg, fp)
                jj = ti // 4
                dst = pj_r[jj][:, ti - 4 * jj, 512 * dc : 512 * (dc + 1)]
                nc.sync.dma_start(dst, stg)

            pj_r = {}   # chunk j -> rearranged partial-output DRAM AP
            pj_ap = {}
            fin_q = []  # (ti, dc) final blocks not yet emitted

            for j in range(NQ):
                sl = slice(512 * j, 512 * (j + 1))

                if j == 0:
                    # first exp needs the mask; tiny load, off the front
                    nc.scalar.dma_start(cm, cmask)

                # ---- stream in chunk j of q, k, v ----
                qb = stream.tile([P, SD, 512], BF16, tag="blk", name="qb")
                if j == 0:
                    # interleave quarters of qb and wq_sb across both queues
                    # so the first matmul starts after ~1MB of DMA
                    for qq in range(4):
                        so = slice(4 * qq, 4 * (qq + 1))
                        nc.sync.dma_start(qb[:, so, :], full[("q", j)][:, so, :])
                        nc.scalar.dma_start(wq_sb[:, so, :], wq_r[:, so, :])
                else:
                    nc.sync.dma_start(qb, full[("q", j)])
                for h in range(n_heads):
                    ps = pp.tile([P, 512], F32, tag="pp", name=f"psq{j}_{h}")
                    for o in range(SD):
                        nc.tensor.matmul(
                            ps, wq_sb[:, o, DH * h : DH * (h + 1)], qb[:, o, :],
                            start=(o == 0), stop=(o == SD - 1),
                        )
                    nc.vector.tensor_copy(xqT[h][:, sl], ps)

                kb = stream.tile([P, SD, 512], BF16, tag="blk", name="kb")
                nc.scalar.dma_start(kb, full[("k", j)])
                if j == 0:
                    nc.sync.dma_start(wk_sb, wk_r)
                for h in range(n_heads):
                    ps = pp.tile([P, 512], F32, tag="pp", name=f"psk{j}_{h}")
                    for o in range(SD):
                        nc.tensor.matmul(
                            ps, wk_sb[:, o, DH * h : DH * (h + 1)], kb[:, o, :],
                            start=(o == 0), stop=(o == SD - 1),
                        )
                    nc.vector.tensor_copy(xkT[h][:, sl], ps)

                vb = stream.tile([P, SD, 512], BF16, tag="blk", name="vb")
                nc.sync.dma_start(vb, full[("v", j)])
                if j == 0:
                    nc.scalar.dma_start(wv_sb, wv_r)
                for st in range(4):
                    ps = pp.tile([P, HD], F32, tag="pp", name=f"psv{j}_{st}")
                    for o in range(SD):
                        nc.tensor.matmul(
                            ps, vb[:, o, P * st : P * (st + 1)], wv_sb[:, o, :],
                            start=(o == 0), stop=(o == SD - 1),
                        )
                    nc.vector.tensor_copy(xv[:, 4 * j + st, :], ps)
                if j == 0:
                    nc.sync.dma_start(wo_sb, wo_r)

                # partial-output staging buffer for this chunk
                pj = pjpool.tile([512, D], BF16, tag="pj", name=f"pj{j}")
                pj_ap[j] = pj
                pj_r[j] = pj.rearrange("(t p) d -> p t d", p=P)

                # ---- causal attention for chunk j, one head at a time ----
                for h in range(n_heads):
                    nkt = 4 * (j + 1)  # causal: only k-tiles at/below diagonal
                    pv = pvp.tile([P, 512], F32, tag="pv", name=f"pv{j}_{h}")
                    dn = dnp.tile([1, 512], F32, tag="dn", name=f"dn{j}_{h}")

                    def make_pt(t, h=h, j=j):
                        d = t - 4 * j
                        c0 = P * d if d > 0 else 0
                        sc = scp.tile([P, 512], F32, tag="sc", name=f"sc{j}_{h}_{t}")
                        nc.tensor.matmul(
                            sc[:, c0:],
                            xkT[h][:, P * t : P * (t + 1)],
                            xqT[h][:, 512 * j + c0 : 512 * (j + 1)],
                            start=True, stop=True,
                        )
                        pt = ptpool.tile([P, 512], BF16, tag="pt",
                                         name=f"pt{j}_{h}_{t}")
                        nc.scalar.activation(
                            pt[:, c0:], sc[:, c0:],
                            mybir.ActivationFunctionType.Exp, scale=inv_sqrt_dh,
                        )
                        if d >= 0:  # mask the block straddling the diagonal
                            nc.vector.tensor_mul(
                                pt[:, c0 : c0 + P], pt[:, c0 : c0 + P], cm
                            )
                        return pt, c0

                    pts = [make_pt(tt) for tt in range(min(pt_ahead, nkt))]
                    for t in range(nkt):
                        pt, c0 = pts[t]
                        if t + pt_ahead < nkt:
                            pts.append(make_pt(t + pt_ahead))
                        nc.tensor.matmul(
                            pv[:, c0:],
                            xv[:, t, DH * h : DH * (h + 1)],
                            pt[:, c0:],
                            start=(t == 0), stop=(t == nkt - 1),
                        )
                        nc.tensor.matmul(
                            dn[:, c0:], ones, pt[:, c0:],
                            start=(t == 0), stop=(t == nkt - 1),
                        )

                    dinv = small.tile([1, 512], F32, tag="dinv")
                    nc.vector.reciprocal(dinv, dn)
                    db = dbpool.tile([P, 512], F32, tag="db")
                    nc.gpsimd.partition_broadcast(db, dinv)
                    nc.vector.tensor_mul(oT[h][:, sl], pv, db)

                    # interleave the previous chunk's output projection
                    # between attention heads to fill exp-latency bubbles
                    for _ in range(4):
                        if fin_q:
                            final_block(*fin_q.pop(0))

                fin_q.extend((ti, dc) for ti in range(4 * j, 4 * (j + 1))
                             for dc in range(ND))
                prev = j - 1
                if prev >= 0:
                    _emit_rs(nc, pj_ap[prev], prev, ojpool, outs)

            while fin_q:
                final_block(*fin_q.pop(0))
            _emit_rs(nc, pj_ap[NQ - 1], NQ - 1, ojpool, outs)

    nc.compile()
    return nc


def _emit_rs(nc, pj, j, ojpool, outs):
    """ReduceScatter chunk j's [512, D] partial across the 4-core group;
    this core keeps rows [128r:128r+128] (r = its rank) and stores them at
    outs[128j : 128j+128]."""
    oj = ojpool.tile([P, 2048], BF16, tag="oj", name=f"oj{j}")
    nc.gpsimd.collective_compute(
        "ReduceScatter", mybir.AluOpType.add,
        replica_groups=GROUPS, ins=[pj.opt()], outs=[oj.opt()],
    )
    nc.gpsimd.dma_start(outs[P * j : P * (j + 1), :], oj)


def make_cmask():
    """cmask[sk_local, sq_local] = 1 if sk_local <= sq_local (bf16)."""
    return np.triu(np.ones((P, P), np.float32)).astype(ml_dtypes.bfloat16)


def run(q, k, v, wq, wk, wv, wo, trace=False, trace_cores=None, **build_kw):
    B, S, D = q.shape
    n_groups = 4  # head groups; 8 cores = B x n_groups
    HD = D // n_groups
    nc = build_nc(S=S, D=D, **build_kw)
    bf = ml_dtypes.bfloat16

    cmask = make_cmask()
    qT = [np.ascontiguousarray(q[b].T).astype(bf) for b in range(B)]
    kT = [np.ascontiguousarray(k[b].T).astype(bf) for b in range(B)]
    vT = [np.ascontiguousarray(v[b].T).astype(bf) for b in range(B)]

    in_maps = []
    for core in range(8):
        b, g = divmod(core, n_groups)
        gs = slice(HD * g, HD * (g + 1))
        m = {
            "qT": qT[b], "kT": kT[b], "vT": vT[b],
            "wq": np.ascontiguousarray(wq[:, gs]).astype(bf),
            "wk": np.ascontiguousarray(wk[:, gs]).astype(bf),
            "wv": np.ascontiguousarray(wv[:, gs]).astype(bf),
            "wo": np.ascontiguousarray(wo[gs, :]).astype(bf),
            "cmask": cmask,
        }
        in_maps.append(m)

    res = run_bass_kernel_spmd(
        nc,
        in_maps,
        core_ids=list(range(8)),
        trace=trace,
        **({"trace_cores": trace_cores} if trace_cores else {}),
    )

    full = np.empty((B, S, D), np.float32)
    for core in range(8):
        b, r = divmod(core, n_groups)
        o = res.results[core]["outs"].astype(np.float32)
        for j in range(S // 512):
            full[b, 512 * j + P * r : 512 * j + P * (r + 1)] = \
                o[P * j : P * (j + 1)]
    return full, res


def kernel(q, k, v, wq, wk, wv, wo):
    full, _ = run(q, k, v, wq, wk, wv, wo)
    return full


# revision 12
# speedup vs baseline: 1.7865x; 1.1818x over previous
"""Causal multi-head attention (B=2, S=2048, D=2048, H=16, Dh=128) on 8 NeuronCores.

Sharding: 8 cores = 2 batches x 4 head-groups; replica groups
[[0,1,2,3],[4,5,6,7]] (one group per batch element). Core (b,g):
  - receives the FULL transposed activations qT/kT/vT of its batch and its
    head-group's weight slices in local DRAM (host-side replication is free:
    the graded metric is NEFF execution time),
  - projects q,k,v against its 512-column slice of wq/wk/wv,
  - runs causal attention for its 4 heads,
  - multiplies by its 512-row slice of wo -> partial [S, D] output,
  - the partial outputs are summed across the 4-core group with per-chunk
    ReduceScatters, so each core downloads only a disjoint [512, D] slice.
Host only reorders rows (no arithmetic beyond dtype cast).

Everything bf16 on the wire and in SBUF; PSUM accumulates fp32.

Layout/scheduling notes:
  - Single j-loop over 512-wide query chunks: project chunk j -> attention
    for chunk j (with the PREVIOUS chunk's wo-projection blocks interleaved
    between heads) -> stage chunk j's partial output. This keeps
    independent PE work available to hide the exp latency, so the PE stays
    warm (HAM K=8/8).
  - Weight loads are just-in-time (wq quartered) so the first projection
    matmul is not queued behind megabytes of DMA.
  - Scores are computed transposed (scoresT[sk, sq]); softmax denominator
    via ones-vector matmul; 1/denom broadcast across partitions with
    gpsimd.partition_broadcast.
  - Causal handling at 128 granularity: for a diagonal tile at offset d,
    columns < 128*d are skipped and only the single 128x128 block that
    straddles the diagonal is masked.
  - score->exp->PV chain pipelined two k-tiles deep.
"""

import math

import ml_dtypes
import numpy as np

import concourse.bass as bass
import concourse.tile as tile
from concourse import bacc, bass_isa, mybir
from concourse.bass_utils import run_bass_kernel_spmd

F32 = mybir.dt.float32
BF16 = mybir.dt.bfloat16

N_HEADS_PER_CORE = 4
DH = 128
P = 128
GROUPS = [[0, 1, 2, 3], [4, 5, 6, 7]]   # per-batch head-group quartets


def build_nc(S=2048, D=2048, n_heads=N_HEADS_PER_CORE, pt_ahead=3):
    """Build the per-core Bass program. Every core runs this same NEFF."""
    HD = n_heads * DH  # head-group width (columns of wq/wk/wv, rows of wo)
    SD = D // P        # contraction chunks for the projections
    NQ = S // 512      # 512-wide sequence chunks
    NT = S // P        # 128-row sequence tiles
    ND = D // 512      # 512-wide model-dim chunks of the output

    inv_sqrt_dh = 1.0 / math.sqrt(DH)

    nc = bacc.Bacc("TRN2", target_bir_lowering=False, debug=False)

    qT = nc.dram_tensor("qT", [D, S], BF16, kind="ExternalInput").ap()
    kT = nc.dram_tensor("kT", [D, S], BF16, kind="ExternalInput").ap()
    vT = nc.dram_tensor("vT", [D, S], BF16, kind="ExternalInput").ap()
    wq = nc.dram_tensor("wq", [D, HD], BF16, kind="ExternalInput").ap()
    wk = nc.dram_tensor("wk", [D, HD], BF16, kind="ExternalInput").ap()
    wv = nc.dram_tensor("wv", [D, HD], BF16, kind="ExternalInput").ap()
    wo = nc.dram_tensor("wo", [HD, D], BF16, kind="ExternalInput").ap()
    outs = nc.dram_tensor("outs", [512, D], BF16, kind="ExternalOutput").ap()
    cmask = nc.dram_tensor("cmask", [P, P], BF16, kind="ExternalInput").ap()

    wq_r = wq.rearrange("(o p) f -> p o f", p=P)
    wk_r = wk.rearrange("(o p) f -> p o f", p=P)
    wv_r = wv.rearrange("(o p) f -> p o f", p=P)
    wo_r = wo.rearrange("(h p) f -> p h f", p=P)

    with tile.TileContext(nc) as tc:
        with (
            tc.tile_pool(name="consts", bufs=1) as consts,
            tc.tile_pool(name="wpool", bufs=1) as wpool,
            tc.tile_pool(name="bigs", bufs=1) as bigs,
            tc.tile_pool(name="stream", bufs=3) as stream,
            tc.tile_pool(name="ptpool", bufs=6) as ptpool,
            tc.tile_pool(name="dspool", bufs=2) as dspool,
            tc.tile_pool(name="dbpool", bufs=2) as dbpool,
            tc.tile_pool(name="ostage", bufs=3) as ostage,
            tc.tile_pool(name="pp", bufs=2, space="PSUM") as pp,
            tc.tile_pool(name="scp", bufs=3, space="PSUM") as scp,
            tc.tile_pool(name="pvp", bufs=2, space="PSUM") as pvp,
            tc.tile_pool(name="pjp", bufs=4, space="DRAM") as pjpool,
            tc.tile_pool(name="ojp", bufs=6, space="DRAM") as ojpool,
        ):
            cm = consts.tile([P, P], BF16)

            wq_sb = wpool.tile([P, SD, HD], BF16, name="wq_sb")
            wk_sb = wpool.tile([P, SD, HD], BF16, name="wk_sb")
            wv_sb = wpool.tile([P, SD, HD], BF16, name="wv_sb")
            wo_sb = wpool.tile([P, n_heads, D], BF16, name="wo_sb")

            full = {}
            for j in range(NQ):
                sj = slice(512 * j, 512 * (j + 1))
                full[("q", j)] = qT.rearrange("(o p) s -> p o s", p=P)[:, :, sj]
                full[("k", j)] = kT.rearrange("(o p) s -> p o s", p=P)[:, :, sj]
                full[("v", j)] = vT.rearrange("(o p) s -> p o s", p=P)[:, :, sj]

            # persistent activations (feature-major, per head)
            xqT = [bigs.tile([P, S], BF16, name=f"xqT{h}") for h in range(n_heads)]
            xkT = [bigs.tile([P, S], BF16, name=f"xkT{h}") for h in range(n_heads)]
            xv = bigs.tile([P, NT, HD], BF16, name="xv")
            oT = [bigs.tile([P, S], BF16, name=f"oT{h}") for h in range(n_heads)]

            def final_block(ti, dc):
                """One [128sq, 512dc] tile of (sum_h oT_h^T @ wo_h) for chunk
                ti//4, staged to the chunk's partial-output DRAM buffer."""
                fp = pp.tile([P, 512], F32, tag="pp", name=f"fp{ti}_{dc}")
                for h in range(n_heads):
                    nc.tensor.matmul(
                        fp,
                        oT[h][:, P * ti : P * (ti + 1)],
                        wo_sb[:, h, 512 * dc : 512 * (dc + 1)],
                        start=(h == 0), stop=(h == n_heads - 1),
                    )
                stg = ostage.tile([P, 512], BF16, tag="ostage")
                nc.vector.tensor_copy(stg, fp)
                jj = ti // 4
                dst = pj_r[jj][:, ti - 4 * jj, 512 * dc : 512 * (dc + 1)]
                nc.sync.dma_start(dst, stg)

            pj_r = {}   # chunk j -> rearranged partial-output DRAM AP
            pj_ap = {}
            ojs = {}    # chunk j -> ReduceScatter output tile
            fin_q = []  # (ti, dc) final blocks not yet emitted

            for j in range(NQ):
                sl = slice(512 * j, 512 * (j + 1))

                if j == 0:
                    # first exp needs the mask; tiny load, off the front
                    nc.scalar.dma_start(cm, cmask)

                # ---- stream in chunk j of q, k, v ----
                qb = stream.tile([P, SD, 512], BF16, tag="blk", name="qb")
                if j == 0:
                    # interleave quarters of qb and wq_sb across both queues
                    # so the first matmul starts after ~1MB of DMA
                    for qq in range(4):
                        so = slice(4 * qq, 4 * (qq + 1))
                        nc.sync.dma_start(qb[:, so, :], full[("q", j)][:, so, :])
                        nc.scalar.dma_start(wq_sb[:, so, :], wq_r[:, so, :])
                else:
                    nc.sync.dma_start(qb, full[("q", j)])
                for h in range(n_heads):
                    ps = pp.tile([P, 512], F32, tag="pp", name=f"psq{j}_{h}")
                    for o in range(SD):
                        nc.tensor.matmul(
                            ps, wq_sb[:, o, DH * h : DH * (h + 1)], qb[:, o, :],
                            start=(o == 0), stop=(o == SD - 1),
                        )
                    nc.vector.tensor_copy(xqT[h][:, sl], ps)

                kb = stream.tile([P, SD, 512], BF16, tag="blk", name="kb")
                nc.scalar.dma_start(kb, full[("k", j)])
                if j == 0:
                    nc.sync.dma_start(wk_sb, wk_r)
                for h in range(n_heads):
                    ps = pp.tile([P, 512], F32, tag="pp", name=f"psk{j}_{h}")
                    for o in range(SD):
                        nc.tensor.matmul(
                            ps, wk_sb[:, o, DH * h : DH * (h + 1)], kb[:, o, :],
                            start=(o == 0), stop=(o == SD - 1),
                        )
                    nc.vector.tensor_copy(xkT[h][:, sl], ps)

                vb = stream.tile([P, SD, 512], BF16, tag="blk", name="vb")
                nc.sync.dma_start(vb, full[("v", j)])
                if j == 0:
                    nc.scalar.dma_start(wv_sb, wv_r)
                for st in range(4):
                    ps = pp.tile([P, HD], F32, tag="pp", name=f"psv{j}_{st}")
                    for o in range(SD):
                        nc.tensor.matmul(
                            ps, vb[:, o, P * st : P * (st + 1)], wv_sb[:, o, :],
                            start=(o == 0), stop=(o == SD - 1),
                        )
                    nc.vector.tensor_copy(xv[:, 4 * j + st, :], ps)
                if j == 0:
                    nc.sync.dma_start(wo_sb, wo_r)

                # partial-output staging buffer for this chunk
                pj = pjpool.tile([512, D], BF16, tag="pj", name=f"pj{j}")
                pj_ap[j] = pj
                pj_r[j] = pj.rearrange("(t p) d -> p t d", p=P)

                # ---- causal attention for chunk j, one head at a time ----
                for h in range(n_heads):
                    nkt = 4 * (j + 1)  # causal: only k-tiles at/below diagonal
                    pv = pvp.tile([P, 512], F32, tag="pv", name=f"pv{j}_{h}")
                    # exp-tile running sum (fp32, on the vector engine) --
                    # keeps the softmax denominator off the PE entirely
                    ptsum = dspool.tile([P, 512], F32, tag="ds",
                                        name=f"ds{j}_{h}")

                    def make_pt(t, h=h, j=j, ptsum=ptsum):
                        d = t - 4 * j
                        c0 = P * d if d > 0 else 0
                        sc = scp.tile([P, 512], F32, tag="sc", name=f"sc{j}_{h}_{t}")
                        nc.tensor.matmul(
                            sc[:, c0:],
                            xkT[h][:, P * t : P * (t + 1)],
                            xqT[h][:, 512 * j + c0 : 512 * (j + 1)],
                            start=True, stop=True,
                        )
                        pt = ptpool.tile([P, 512], BF16, tag="pt",
                                         name=f"pt{j}_{h}_{t}")
                        nc.scalar.activation(
                            pt[:, c0:], sc[:, c0:],
                            mybir.ActivationFunctionType.Exp, scale=inv_sqrt_dh,
                        )
                        if d >= 0:  # mask the block straddling the diagonal
                            nc.vector.tensor_mul(
                                pt[:, c0 : c0 + P], pt[:, c0 : c0 + P], cm
                            )
                        if t == 0:
                            nc.vector.tensor_copy(ptsum, pt)
                        else:
                            nc.vector.tensor_add(
                                ptsum[:, c0:], ptsum[:, c0:], pt[:, c0:]
                            )
                        return pt, c0

                    pts = [make_pt(tt) for tt in range(min(pt_ahead, nkt))]
                    for t in range(nkt):
                        pt, c0 = pts[t]
                        if t + pt_ahead < nkt:
                            pts.append(make_pt(t + pt_ahead))
                        nc.tensor.matmul(
                            pv[:, c0:],
                            xv[:, t, DH * h : DH * (h + 1)],
                            pt[:, c0:],
                            start=(t == 0), stop=(t == nkt - 1),
                        )

                    # denominator: all-partition sum of ptsum, broadcast to
                    # every partition (gpsimd), then 1/x and the scale-mul
                    db = dbpool.tile([P, 512], F32, tag="db")
                    nc.gpsimd.partition_all_reduce(
                        db, ptsum, channels=P, reduce_op=bass_isa.ReduceOp.add
                    )
                    dbi = dbpool.tile([P, 512], F32, tag="db")
                    nc.vector.reciprocal(dbi, db)
                    nc.vector.tensor_mul(oT[h][:, sl], pv, dbi)

                    # interleave the previous chunk's output projection
                    # between attention heads to fill exp-latency bubbles
                    for _ in range(8):
                        if fin_q:
                            final_block(*fin_q.pop(0))
                    # chunk j-1's 16 blocks have all been staged after the
                    # h==1 slot; trigger its ReduceScatter right away so it
                    # overlaps the rest of this chunk's attention. No outs
                    # DMA sits ahead of it on the gpsimd queue.
                    if h == 1 and j >= 1:
                        ojs[j - 1] = _rs(nc, pj_ap[j - 1], ojpool, f"oj{j-1}")

                fin_q.extend((ti, dc) for ti in range(4 * j, 4 * (j + 1))
                             for dc in range(ND))

            # drain chunk 3: two RS halves so the exposed tail is one
            # half-sized collective, with outs DMAs on the sync queue
            for _ in range(8):
                final_block(*fin_q.pop(0))
            oj3a = _rs(nc, pj_ap[NQ - 1][0:256, :], ojpool, "oj3a")
            while fin_q:
                final_block(*fin_q.pop(0))
            oj3b = _rs(nc, pj_ap[NQ - 1][256:512, :], ojpool, "oj3b")
            for jj in range(NQ - 1):
                nc.sync.dma_start(outs[P * jj : P * (jj + 1), :], ojs[jj])
            nc.sync.dma_start(outs[384:448, :], oj3a)
            nc.sync.dma_start(outs[448:512, :], oj3b)

    nc.compile()
    return nc


def _rs(nc, pj, ojpool, name):
    """ReduceScatter a [R, D] partial across the 4-core group; this core
    keeps rows [R//4*r : R//4*(r+1)] (r = its rank)."""
    rows = pj.shape[0]
    oj = ojpool.tile([rows // 4, pj.shape[1]], BF16, tag="oj", name=name)
    nc.gpsimd.collective_compute(
        "ReduceScatter", mybir.AluOpType.add,
        replica_groups=GROUPS, ins=[pj.opt()], outs=[oj.opt()],
    )
    return oj


def make_cmask():
    """cmask[sk_local, sq_local] = 1 if sk_local <= sq_local (bf16)."""
    return np.triu(np.ones((P, P), np.float32)).astype(ml_dtypes.bfloat16)


def run(q, k, v, wq, wk, wv, wo, trace=False, trace_cores=None, **build_kw):
    B, S, D = q.shape
    n_groups = 4  # head groups; 8 cores = B x n_groups
    HD = D // n_groups
    nc = build_nc(S=S, D=D, **build_kw)
    bf = ml_dtypes.bfloat16

    cmask = make_cmask()
    qT = [np.ascontiguousarray(q[b].T).astype(bf) for b in range(B)]
    kT = [np.ascontiguousarray(k[b].T).astype(bf) for b in range(B)]
    vT = [np.ascontiguousarray(v[b].T).astype(bf) for b in range(B)]

    in_maps = []
    for core in range(8):
        b, g = divmod(core, n_groups)
        gs = slice(HD * g, HD * (g + 1))
        m = {
            "qT": qT[b], "kT": kT[b], "vT": vT[b],
            "wq": np.ascontiguousarray(wq[:, gs]).astype(bf),
            "wk": np.ascontiguousarray(wk[:, gs]).astype(bf),
            "wv": np.ascontiguousarray(wv[:, gs]).astype(bf),
            "wo": np.ascontiguousarray(wo[gs, :]).astype(bf),
            "cmask": cmask,
        }
        in_maps.append(m)

    res = run_bass_kernel_spmd(
        nc,
        in_maps,
        core_ids=list(range(8)),
        trace=trace,
        **({"trace_cores": trace_cores} if trace_cores else {}),
    )

    full = np.empty((B, S, D), np.float32)
    for core in range(8):
        b, r = divmod(core, n_groups)
        o = res.results[core]["outs"].astype(np.float32)
        for j in range(S // 512 - 1):
            full[b, 512 * j + P * r : 512 * j + P * (r + 1)] = \
                o[P * j : P * (j + 1)]
        # last chunk arrives as two [256,D] ReduceScatters (64 rows each)
        full[b, 1536 + 64 * r : 1536 + 64 * (r + 1)] = o[384:448]
        full[b, 1792 + 64 * r : 1792 + 64 * (r + 1)] = o[448:512]
    return full, res


def kernel(q, k, v, wq, wk, wv, wo):
    full, _ = run(q, k, v, wq, wk, wv, wo)
    return full


# revision 13
# speedup vs baseline: 1.7907x; 1.0024x over previous
"""Causal multi-head attention (B=2, S=2048, D=2048, H=16, Dh=128) on 8 NeuronCores.

Sharding: 8 cores = 2 batches x 4 head-groups; replica groups
[[0,1,2,3],[4,5,6,7]] (one group per batch element). Core (b,g):
  - receives the FULL transposed activations qT/kT/vT of its batch and its
    head-group's weight slices in local DRAM (host-side replication is free:
    the graded metric is NEFF execution time),
  - projects q,k,v against its 512-column slice of wq/wk/wv,
  - runs causal attention for its 4 heads,
  - multiplies by its 512-row slice of wo -> partial [S, D] output,
  - the partial outputs are summed across the 4-core group with
    ReduceScatters (one per 512-query chunk; the last chunk in two halves
    so the exposed tail is one half-sized collective), each core keeping a
    disjoint row slice.
Host only reorders rows (no arithmetic beyond dtype cast).

Everything bf16 on the wire and in SBUF; PSUM accumulates fp32.

Layout/scheduling notes:
  - Single j-loop over 512-wide query chunks: project chunk j -> attention
    for chunk j (with the PREVIOUS chunk's wo-projection blocks interleaved
    between heads) -> stage chunk j's partial output. Keeps independent PE
    work available so the PE never idles (p-state stays high).
  - Chunk j+1's q/k/v DMAs are issued at head 1 of chunk j's attention, so
    projections never wait on HBM.
  - Scores are computed transposed (scoresT[sk, sq]); softmax denominator
    accumulated on the vector engine (exp-tile running sum), reduced across
    partitions on gpsimd, inverted with the fast DVE reciprocal.
  - Causal handling at 128 granularity: for a tile straddling the diagonal,
    columns left of the tile are skipped and the single 128x128 straddling
    block is masked.
  - score->exp->PV chain pipelined four k-tiles deep.
  - PSUM->SBUF copies of the output projection go through the scalar
    engine (Copy activation) to keep the vector engine off the critical
    path.
"""

import math

import ml_dtypes
import numpy as np

import concourse.bass as bass
import concourse.tile as tile
from concourse import bacc, bass_isa, mybir
from concourse.bass_utils import run_bass_kernel_spmd

F32 = mybir.dt.float32
BF16 = mybir.dt.bfloat16

N_HEADS_PER_CORE = 4
DH = 128
P = 128
GROUPS = [[0, 1, 2, 3], [4, 5, 6, 7]]   # per-batch head-group quartets


def build_nc(S=2048, D=2048, n_heads=N_HEADS_PER_CORE, pt_ahead=4):
    """Build the per-core Bass program. Every core runs this same NEFF."""
    HD = n_heads * DH  # head-group width (columns of wq/wk/wv, rows of wo)
    SD = D // P        # contraction chunks for the projections
    NQ = S // 512      # 512-wide sequence chunks
    NT = S // P        # 128-row sequence tiles
    ND = D // 512      # 512-wide model-dim chunks of the output

    inv_sqrt_dh = 1.0 / math.sqrt(DH)

    nc = bacc.Bacc("TRN2", target_bir_lowering=False, debug=False)

    qT = nc.dram_tensor("qT", [D, S], BF16, kind="ExternalInput").ap()
    kT = nc.dram_tensor("kT", [D, S], BF16, kind="ExternalInput").ap()
    vT = nc.dram_tensor("vT", [D, S], BF16, kind="ExternalInput").ap()
    wq = nc.dram_tensor("wq", [D, HD], BF16, kind="ExternalInput").ap()
    wk = nc.dram_tensor("wk", [D, HD], BF16, kind="ExternalInput").ap()
    wv = nc.dram_tensor("wv", [D, HD], BF16, kind="ExternalInput").ap()
    wo = nc.dram_tensor("wo", [HD, D], BF16, kind="ExternalInput").ap()
    outs = nc.dram_tensor("outs", [512, D], BF16, kind="ExternalOutput").ap()
    cmask = nc.dram_tensor("cmask", [P, P], BF16, kind="ExternalInput").ap()

    wq_r = wq.rearrange("(o p) f -> p o f", p=P)
    wk_r = wk.rearrange("(o p) f -> p o f", p=P)
    wv_r = wv.rearrange("(o p) f -> p o f", p=P)
    wo_r = wo.rearrange("(h p) f -> p h f", p=P)

    with tile.TileContext(nc) as tc:
        with (
            tc.tile_pool(name="consts", bufs=1) as consts,
            tc.tile_pool(name="wpool", bufs=1) as wpool,
            tc.tile_pool(name="bigs", bufs=1) as bigs,
            tc.tile_pool(name="stream", bufs=3) as stream,
            tc.tile_pool(name="ptpool", bufs=6) as ptpool,
            tc.tile_pool(name="dspool", bufs=2) as dspool,
            tc.tile_pool(name="dbpool", bufs=2) as dbpool,
            tc.tile_pool(name="ostage", bufs=3) as ostage,
            tc.tile_pool(name="pp", bufs=2, space="PSUM") as pp,
            tc.tile_pool(name="scp", bufs=4, space="PSUM") as scp,
            tc.tile_pool(name="pvp", bufs=2, space="PSUM") as pvp,
            tc.tile_pool(name="pjp", bufs=4, space="DRAM") as pjpool,
            tc.tile_pool(name="ojp", bufs=6, space="DRAM") as ojpool,
        ):
            cm = consts.tile([P, P], BF16)

            wq_sb = wpool.tile([P, SD, HD], BF16, name="wq_sb")
            wk_sb = wpool.tile([P, SD, HD], BF16, name="wk_sb")
            wv_sb = wpool.tile([P, SD, HD], BF16, name="wv_sb")
            wo_sb = wpool.tile([P, n_heads, D], BF16, name="wo_sb")

            full = {}
            for j in range(NQ):
                sj = slice(512 * j, 512 * (j + 1))
                full[("q", j)] = qT.rearrange("(o p) s -> p o s", p=P)[:, :, sj]
                full[("k", j)] = kT.rearrange("(o p) s -> p o s", p=P)[:, :, sj]
                full[("v", j)] = vT.rearrange("(o p) s -> p o s", p=P)[:, :, sj]

            # persistent activations (feature-major, per head)
            xqT = [bigs.tile([P, S], BF16, name=f"xqT{h}") for h in range(n_heads)]
            xkT = [bigs.tile([P, S], BF16, name=f"xkT{h}") for h in range(n_heads)]
            xv = bigs.tile([P, NT, HD], BF16, name="xv")
            oT = [bigs.tile([P, S], BF16, name=f"oT{h}") for h in range(n_heads)]

            def final_block(ti, dc):
                """One [128sq, 512dc] tile of (sum_h oT_h^T @ wo_h) for chunk
                ti//4, staged to the chunk's partial-output DRAM buffer."""
                fp = pp.tile([P, 512], F32, tag="pp", name=f"fp{ti}_{dc}")
                for h in range(n_heads):
                    nc.tensor.matmul(
                        fp,
                        oT[h][:, P * ti : P * (ti + 1)],
                        wo_sb[:, h, 512 * dc : 512 * (dc + 1)],
                        start=(h == 0), stop=(h == n_heads - 1),
                    )
                stg = ostage.tile([P, 512], BF16, tag="ostage")
                nc.scalar.activation(stg, fp, mybir.ActivationFunctionType.Copy)
                jj = ti // 4
                dst = pj_r[jj][:, ti - 4 * jj, 512 * dc : 512 * (dc + 1)]
                nc.sync.dma_start(dst, stg)

            def prefetch(jn):
                """Issue chunk jn's activation DMAs (called from inside the
                previous chunk's attention, where these queues are idle)."""
                qb = stream.tile([P, SD, 512], BF16, tag="blk", name="qb")
                kb = stream.tile([P, SD, 512], BF16, tag="blk", name="kb")
                vb = stream.tile([P, SD, 512], BF16, tag="blk", name="vb")
                nc.sync.dma_start(qb, full[("q", jn)])
                nc.sync.dma_start(kb, full[("k", jn)])
                nc.scalar.dma_start(vb, full[("v", jn)])
                return qb, kb, vb

            pj_r = {}   # chunk j -> rearranged partial-output DRAM AP
            pj_ap = {}
            ojs = {}    # chunk j -> ReduceScatter output tile
            fin_q = []  # (ti, dc) final blocks not yet emitted
            cur = {}    # chunk j's streamed qb/kb/vb

            def attn_heads(j, q0, qw, on_h1=None):
                """Causal attention for queries [512j+q0, 512j+q0+qw), all
                heads, interleaving queued wo-projection blocks."""
                Q0 = 512 * j + q0
                sl = slice(Q0, Q0 + qw)
                nkt = (Q0 + qw) // P
                for h in range(n_heads):
                    pv = pvp.tile([P, qw], F32, tag="pv", name=f"pv{j}_{q0}_{h}")
                    # exp-tile running sum (fp32, vector engine) -- keeps the
                    # softmax denominator off the PE entirely
                    ptsum = dspool.tile([P, qw], F32, tag="ds",
                                        name=f"ds{j}_{q0}_{h}")

                    def make_pt(t, h=h, ptsum=ptsum):
                        off = P * t - Q0
                        c0 = max(0, off)
                        sc = scp.tile([P, qw], F32, tag="sc",
                                      name=f"sc{j}_{q0}_{h}_{t}")
                        nc.tensor.matmul(
                            sc[:, c0:],
                            xkT[h][:, P * t : P * (t + 1)],
                            xqT[h][:, Q0 + c0 : Q0 + qw],
                            start=True, stop=True,
                        )
                        pt = ptpool.tile([P, qw], BF16, tag="pt",
                                         name=f"pt{j}_{q0}_{h}_{t}")
                        nc.scalar.activation(
                            pt[:, c0:], sc[:, c0:],
                            mybir.ActivationFunctionType.Exp, scale=inv_sqrt_dh,
                        )
                        if off >= 0:  # mask the block straddling the diagonal
                            nc.vector.tensor_mul(
                                pt[:, c0 : c0 + P], pt[:, c0 : c0 + P], cm
                            )
                        if t == 0:
                            nc.vector.tensor_copy(ptsum, pt)
                        else:
                            nc.vector.tensor_add(
                                ptsum[:, c0:], ptsum[:, c0:], pt[:, c0:]
                            )
                        return pt, c0

                    pts = [make_pt(tt) for tt in range(min(pt_ahead, nkt))]
                    for t in range(nkt):
                        pt, c0 = pts[t]
                        if t + pt_ahead < nkt:
                            pts.append(make_pt(t + pt_ahead))
                        nc.tensor.matmul(
                            pv[:, c0:],
                            xv[:, t, DH * h : DH * (h + 1)],
                            pt[:, c0:],
                            start=(t == 0), stop=(t == nkt - 1),
                        )

                    # denominator: all-partition sum of ptsum broadcast to
                    # every partition (gpsimd), fast 1/x, then the scale-mul
                    db = dbpool.tile([P, qw], F32, tag="db")
                    nc.gpsimd.partition_all_reduce(
                        db, ptsum, channels=P, reduce_op=bass_isa.ReduceOp.add
                    )
                    dbi = dbpool.tile([P, qw], F32, tag="db")
                    nc.vector.reciprocal_approx_fast(dbi, db)
                    nc.vector.tensor_mul(oT[h][:, sl], pv, dbi)

                    # interleave the previous chunk's output projection
                    # between attention heads to fill exp-latency bubbles
                    for _ in range(8):
                        if fin_q:
                            final_block(*fin_q.pop(0))
                    if h == 1 and on_h1 is not None:
                        on_h1()

            # ---- initial loads: chunk 0 interleaved with weights so the
            # first projection matmul starts after ~0.5MB of DMA ----
            nc.scalar.dma_start(cm, cmask)
            qb0 = stream.tile([P, SD, 512], BF16, tag="blk", name="qb")
            kb0 = stream.tile([P, SD, 512], BF16, tag="blk", name="kb")
            vb0 = stream.tile([P, SD, 512], BF16, tag="blk", name="vb")
            for e in range(8):
                so = slice(2 * e, 2 * (e + 1))
                nc.sync.dma_start(qb0[:, so, :], full[("q", 0)][:, so, :])
                nc.scalar.dma_start(wq_sb[:, so, :], wq_r[:, so, :])
            for qq in range(4):
                so = slice(4 * qq, 4 * (qq + 1))
                nc.sync.dma_start(kb0[:, so, :], full[("k", 0)][:, so, :])
                nc.scalar.dma_start(wk_sb[:, so, :], wk_r[:, so, :])
            nc.sync.dma_start(vb0, full[("v", 0)])
            nc.scalar.dma_start(wv_sb, wv_r)
            nc.sync.dma_start(wo_sb, wo_r)
            cur = (qb0, kb0, vb0)

            for j in range(NQ):
                sl = slice(512 * j, 512 * (j + 1))
                qb, kb, vb = cur

                # ---- project chunk j ----
                for h in range(n_heads):
                    ps = pp.tile([P, 512], F32, tag="pp", name=f"psq{j}_{h}")
                    for o in range(SD):
                        nc.tensor.matmul(
                            ps, wq_sb[:, o, DH * h : DH * (h + 1)], qb[:, o, :],
                            start=(o == 0), stop=(o == SD - 1),
                        )
                    nc.vector.tensor_copy(xqT[h][:, sl], ps)

                for h in range(n_heads):
                    ps = pp.tile([P, 512], F32, tag="pp", name=f"psk{j}_{h}")
                    for o in range(SD):
                        nc.tensor.matmul(
                            ps, wk_sb[:, o, DH * h : DH * (h + 1)], kb[:, o, :],
                            start=(o == 0), stop=(o == SD - 1),
                        )
                    nc.vector.tensor_copy(xkT[h][:, sl], ps)

                for st in range(4):
                    ps = pp.tile([P, HD], F32, tag="pp", name=f"psv{j}_{st}")
                    for o in range(SD):
                        nc.tensor.matmul(
                            ps, vb[:, o, P * st : P * (st + 1)], wv_sb[:, o, :],
                            start=(o == 0), stop=(o == SD - 1),
                        )
                    nc.vector.tensor_copy(xv[:, 4 * j + st, :], ps)

                # partial-output staging buffer for this chunk
                pj = pjpool.tile([512, D], BF16, tag="pj", name=f"pj{j}")
                pj_ap[j] = pj
                pj_r[j] = pj.rearrange("(t p) d -> p t d", p=P)

                def on_h1(j=j):
                    if j >= 1:
                        ojs[j - 1] = _rs(nc, pj_ap[j - 1], ojpool, f"oj{j-1}")
                    if j + 1 < NQ:
                        cur_next[0] = prefetch(j + 1)

                cur_next = [None]
                if j < NQ - 1:
                    attn_heads(j, 0, 512, on_h1)
                    fin_q.extend((ti, dc) for ti in range(4 * j, 4 * (j + 1))
                                 for dc in range(ND))
                    cur = cur_next[0]
                else:
                    # last chunk: two half-attentions so its ReduceScatter
                    # splits into an early (hidden) half and a small tail
                    attn_heads(j, 0, 256, on_h1)
                    for ti in (4 * j, 4 * j + 1):
                        for dc in range(ND):
                            final_block(ti, dc)
                    oj3a = _rs(nc, pj_ap[j][0:256, :], ojpool, "oj3a")
                    attn_heads(j, 256, 256)
                    for ti in (4 * j + 2, 4 * j + 3):
                        for dc in range(ND):
                            final_block(ti, dc)
                    oj3b = _rs(nc, pj_ap[j][256:512, :], ojpool, "oj3b")

            for jj in range(NQ - 1):
                nc.sync.dma_start(outs[P * jj : P * (jj + 1), :], ojs[jj])
            nc.sync.dma_start(outs[384:448, :], oj3a)
            nc.sync.dma_start(outs[448:512, :], oj3b)

    nc.compile()
    return nc


def _rs(nc, pj, ojpool, name):
    """ReduceScatter a [R, D] partial across the 4-core group; this core
    keeps rows [R//4*r : R//4*(r+1)] (r = its rank)."""
    rows = pj.shape[0]
    oj = ojpool.tile([rows // 4, pj.shape[1]], BF16, tag="oj", name=name)
    nc.gpsimd.collective_compute(
        "ReduceScatter", mybir.AluOpType.add,
        replica_groups=GROUPS, ins=[pj.opt()], outs=[oj.opt()],
    )
    return oj


def make_cmask():
    """cmask[sk_local, sq_local] = 1 if sk_local <= sq_local (bf16)."""
    return np.triu(np.ones((P, P), np.float32)).astype(ml_dtypes.bfloat16)


def run(q, k, v, wq, wk, wv, wo, trace=False, trace_cores=None, **build_kw):
    B, S, D = q.shape
    n_groups = 4  # head groups; 8 cores = B x n_groups
    HD = D // n_groups
    nc = build_nc(S=S, D=D, **build_kw)
    bf = ml_dtypes.bfloat16

    cmask = make_cmask()
    qT = [np.ascontiguousarray(q[b].T).astype(bf) for b in range(B)]
    kT = [np.ascontiguousarray(k[b].T).astype(bf) for b in range(B)]
    vT = [np.ascontiguousarray(v[b].T).astype(bf) for b in range(B)]

    in_maps = []
    for core in range(8):
        b, g = divmod(core, n_groups)
        gs = slice(HD * g, HD * (g + 1))
        m = {
            "qT": qT[b], "kT": kT[b], "vT": vT[b],
            "wq": np.ascontiguousarray(wq[:, gs]).astype(bf),
            "wk": np.ascontiguousarray(wk[:, gs]).astype(bf),
            "wv": np.ascontiguousarray(wv[:, gs]).astype(bf),
            "wo": np.ascontiguousarray(wo[gs, :]).astype(bf),
            "cmask": cmask,
        }
        in_maps.append(m)

    res = run_bass_kernel_spmd(
        nc,
        in_maps,
        core_ids=list(range(8)),
        trace=trace,
        **({"trace_cores": trace_cores} if trace_cores else {}),
    )

    full = np.empty((B, S, D), np.float32)
    for core in range(8):
        b, r = divmod(core, n_groups)
        o = res.results[core]["outs"].astype(np.float32)
        for j in range(S // 512 - 1):
            full[b, 512 * j + P * r : 512 * j + P * (r + 1)] = \
                o[P * j : P * (j + 1)]
        # last chunk arrives as two [256,D] ReduceScatters (64 rows each)
        full[b, 1536 + 64 * r : 1536 + 64 * (r + 1)] = o[384:448]
        full[b, 1792 + 64 * r : 1792 + 64 * (r + 1)] = o[448:512]
    return full, res


def kernel(q, k, v, wq, wk, wv, wo):
    full, _ = run(q, k, v, wq, wk, wv, wo)
    return full


# revision 18
# speedup vs baseline: 1.9461x; 1.0868x over previous
"""Causal multi-head attention (B=2, S=2048, D=2048, H=16, Dh=128) on 8 NeuronCores.

Sharding: 8 cores = 2 batches x 4 head-groups; replica groups
[[0,1,2,3],[4,5,6,7]] (one group per batch element). Core (b,g):
  - receives the FULL transposed activations qT/kT/vT of its batch and its
    head-group's weight slices in local DRAM (host-side replication is free:
    the graded metric is NEFF execution time),
  - projects q,k,v against its 512-column slice of wq/wk/wv,
  - runs causal attention for its 4 heads,
  - multiplies by its 512-row slice of wo -> partial [S, D] output,
  - the partial outputs are summed across the 4-core group with
    ReduceScatters (one per 512-query chunk; the last chunk in two halves
    so the exposed tail is one half-sized collective), each core keeping a
    disjoint row slice.
Host only reorders rows (no arithmetic beyond dtype cast).

Everything bf16 on the wire and in SBUF; PSUM accumulates fp32.

Layout/scheduling notes:
  - Single j-loop over 512-wide query chunks: project chunk j -> attention
    for chunk j (with the PREVIOUS chunk's wo-projection blocks interleaved
    between heads) -> stage chunk j's partial output. Keeps independent PE
    work available so the PE never idles (p-state stays high).
  - Chunk j+1's q/k/v DMAs are issued at head 1 of chunk j's attention, so
    projections never wait on HBM.
  - Scores are computed transposed (scoresT[sk, sq]); softmax denominator
    accumulated on the vector engine (exp-tile running sum), reduced across
    partitions on gpsimd, inverted with the fast DVE reciprocal.
  - Causal handling at 128 granularity: for a tile straddling the diagonal,
    columns left of the tile are skipped and the single 128x128 straddling
    block is masked.
  - score->exp->PV chain pipelined four k-tiles deep.
  - PSUM->SBUF copies of the output projection go through the scalar
    engine (Copy activation) to keep the vector engine off the critical
    path.
"""

import math

import ml_dtypes
import numpy as np

import concourse.bass as bass
import concourse.tile as tile
from concourse import bacc, bass_isa, mybir
from concourse.bass_utils import run_bass_kernel_spmd

F32 = mybir.dt.float32
BF16 = mybir.dt.bfloat16

N_HEADS_PER_CORE = 4
DH = 128
P = 128
GROUPS = [[0, 1, 2, 3], [4, 5, 6, 7]]   # per-batch head-group quartets


def build_nc(S=2048, D=2048, n_heads=N_HEADS_PER_CORE, pt_ahead=4):
    """Build the per-core Bass program. Every core runs this same NEFF."""
    HD = n_heads * DH  # head-group width (columns of wq/wk/wv, rows of wo)
    SD = D // P        # contraction chunks for the projections
    NQ = S // 512      # 512-wide sequence chunks
    NT = S // P        # 128-row sequence tiles
    ND = D // 512      # 512-wide model-dim chunks of the output

    inv_sqrt_dh = 1.0 / math.sqrt(DH)

    nc = bacc.Bacc("TRN2", target_bir_lowering=False, debug=False)

    qT = nc.dram_tensor("qT", [D, S], BF16, kind="ExternalInput").ap()
    kT = nc.dram_tensor("kT", [D, S], BF16, kind="ExternalInput").ap()
    vT = nc.dram_tensor("vT", [D, S], BF16, kind="ExternalInput").ap()
    wq = nc.dram_tensor("wq", [D, HD], BF16, kind="ExternalInput").ap()
    wk = nc.dram_tensor("wk", [D, HD], BF16, kind="ExternalInput").ap()
    wv = nc.dram_tensor("wv", [D, HD], BF16, kind="ExternalInput").ap()
    wo = nc.dram_tensor("wo", [HD, D], BF16, kind="ExternalInput").ap()
    outs = nc.dram_tensor("outs", [512, D], BF16, kind="ExternalOutput").ap()
    cmask = nc.dram_tensor("cmask", [P, P], BF16, kind="ExternalInput").ap()

    wq_r = wq.rearrange("(o p) f -> p o f", p=P)
    wk_r = wk.rearrange("(o p) f -> p o f", p=P)
    wv_r = wv.rearrange("(o p) f -> p o f", p=P)
    wo_r = wo.rearrange("(h p) f -> p h f", p=P)

    with tile.TileContext(nc) as tc:
        with (
            tc.tile_pool(name="consts", bufs=1) as consts,
            tc.tile_pool(name="wpool", bufs=1) as wpool,
            tc.tile_pool(name="bigs", bufs=1) as bigs,
            tc.tile_pool(name="stream", bufs=3) as stream,
            tc.tile_pool(name="ptpool", bufs=6) as ptpool,
            tc.tile_pool(name="dspool", bufs=2) as dspool,
            tc.tile_pool(name="dbpool", bufs=2) as dbpool,
            tc.tile_pool(name="ostage", bufs=8) as ostage,
            tc.tile_pool(name="pp", bufs=2, space="PSUM") as pp,
            tc.tile_pool(name="scp", bufs=4, space="PSUM") as scp,
            tc.tile_pool(name="pvp", bufs=2, space="PSUM") as pvp,
            tc.tile_pool(name="pjp", bufs=4, space="DRAM") as pjpool,
            tc.tile_pool(name="ojp", bufs=6, space="DRAM") as ojpool,
        ):
            cm = consts.tile([P, P], BF16)

            wq_sb = wpool.tile([P, SD, HD], BF16, name="wq_sb")
            wk_sb = wpool.tile([P, SD, HD], BF16, name="wk_sb")
            wv_sb = wpool.tile([P, SD, HD], BF16, name="wv_sb")
            wo_sb = wpool.tile([P, n_heads, D], BF16, name="wo_sb")

            full = {}
            for j in range(NQ):
                sj = slice(512 * j, 512 * (j + 1))
                full[("q", j)] = qT.rearrange("(o p) s -> p o s", p=P)[:, :, sj]
                full[("k", j)] = kT.rearrange("(o p) s -> p o s", p=P)[:, :, sj]
                full[("v", j)] = vT.rearrange("(o p) s -> p o s", p=P)[:, :, sj]

            # persistent activations (feature-major, per head)
            xqT = [bigs.tile([P, S], BF16, name=f"xqT{h}") for h in range(n_heads)]
            xkT = [bigs.tile([P, S], BF16, name=f"xkT{h}") for h in range(n_heads)]
            xv = bigs.tile([P, NT, HD], BF16, name="xv")
            oT = [bigs.tile([P, S], BF16, name=f"oT{h}") for h in range(n_heads)]

            def final_block(ti, dc):
                """One [128sq, 512dc] tile of (sum_h oT_h^T @ wo_h) for chunk
                ti//4, staged to the chunk's partial-output DRAM buffer."""
                fp = pp.tile([P, 512], F32, tag="pp", name=f"fp{ti}_{dc}")
                for h in range(n_heads):
                    nc.tensor.matmul(
                        fp,
                        oT[h][:, P * ti : P * (ti + 1)],
                        wo_sb[:, h, 512 * dc : 512 * (dc + 1)],
                        start=(h == 0), stop=(h == n_heads - 1),
                    )
                stg = ostage.tile([P, 512], BF16, tag="ostage")
                nc.scalar.activation(stg, fp, mybir.ActivationFunctionType.Copy)
                jj = ti // 4
                dst = pj_r[jj][:, ti - 4 * jj, 512 * dc : 512 * (dc + 1)]
                nc.sync.dma_start(dst, stg)

            def prefetch(jn, part):
                """Issue one of chunk jn's activation DMAs (called from
                inside the previous chunk's attention, one tensor per head
                slot so the bursts don't collide with the ReduceScatter)."""
                t = stream.tile([P, SD, 512], BF16, tag="blk",
                                name="qkv"[part] + "b")
                nc.sync.dma_start(t, full[("qkv"[part], jn)])
                return t

            pj_r = {}   # chunk j -> rearranged partial-output DRAM AP
            pj_ap = {}
            ojs = {}    # chunk j -> ReduceScatter output tile
            fin_q = []  # (ti, dc) final blocks not yet emitted
            cur = {}    # chunk j's streamed qb/kb/vb

            def attn_heads(j, q0, qw, on_head=None):
                """Causal attention for queries [512j+q0, 512j+q0+qw), all
                heads, interleaving queued wo-projection blocks."""
                Q0 = 512 * j + q0
                sl = slice(Q0, Q0 + qw)
                nkt = (Q0 + qw) // P
                for h in range(n_heads):
                    pv = pvp.tile([P, qw], F32, tag="pv", name=f"pv{j}_{q0}_{h}")
                    # exp-tile running sum (fp32, vector engine) -- keeps the
                    # softmax denominator off the PE entirely
                    ptsum = dspool.tile([P, qw], F32, tag="ds",
                                        name=f"ds{j}_{q0}_{h}")

                    def make_pt(t, h=h, ptsum=ptsum):
                        off = P * t - Q0
                        c0 = max(0, off)
                        sc = scp.tile([P, qw], F32, tag="sc",
                                      name=f"sc{j}_{q0}_{h}_{t}")
                        nc.tensor.matmul(
                            sc[:, c0:],
                            xkT[h][:, P * t : P * (t + 1)],
                            xqT[h][:, Q0 + c0 : Q0 + qw],
                            start=True, stop=True,
                        )
                        pt = ptpool.tile([P, qw], BF16, tag="pt",
                                         name=f"pt{j}_{q0}_{h}_{t}")
                        nc.scalar.activation(
                            pt[:, c0:], sc[:, c0:],
                            mybir.ActivationFunctionType.Exp, scale=inv_sqrt_dh,
                        )
                        if off >= 0:  # mask the block straddling the diagonal
                            nc.vector.tensor_mul(
                                pt[:, c0 : c0 + P], pt[:, c0 : c0 + P], cm
                            )
                        if t == 0:
                            nc.vector.tensor_copy(ptsum, pt)
                        else:
                            nc.vector.tensor_add(
                                ptsum[:, c0:], ptsum[:, c0:], pt[:, c0:]
                            )
                        return pt, c0

                    pts = [make_pt(tt) for tt in range(min(pt_ahead, nkt))]
                    for t in range(nkt):
                        pt, c0 = pts[t]
                        if t + pt_ahead < nkt:
                            pts.append(make_pt(t + pt_ahead))
                        nc.tensor.matmul(
                            pv[:, c0:],
                            xv[:, t, DH * h : DH * (h + 1)],
                            pt[:, c0:],
                            start=(t == 0), stop=(t == nkt - 1),
                        )

                    # denominator: all-partition sum of ptsum broadcast to
                    # every partition (gpsimd), fast 1/x, then the scale-mul
                    db = dbpool.tile([P, qw], F32, tag="db")
                    nc.gpsimd.partition_all_reduce(
                        db, ptsum, channels=P, reduce_op=bass_isa.ReduceOp.add
                    )
                    dbi = dbpool.tile([P, qw], F32, tag="db")
                    nc.vector.reciprocal_approx_fast(dbi, db)
                    nc.vector.tensor_mul(oT[h][:, sl], pv, dbi)

                    # interleave the previous chunk's output projection
                    # between attention heads to fill exp-latency bubbles
                    for _ in range(8):
                        if fin_q:
                            final_block(*fin_q.pop(0))
                    if on_head is not None:
                        on_head(h)

            # ---- initial loads: chunk 0 interleaved with weights so the
            # first projection matmul starts after ~0.5MB of DMA ----
            nc.scalar.dma_start(cm, cmask)
            qb0 = stream.tile([P, SD, 512], BF16, tag="blk", name="qb")
            kb0 = stream.tile([P, SD, 512], BF16, tag="blk", name="kb")
            vb0 = stream.tile([P, SD, 512], BF16, tag="blk", name="vb")
            for e in range(8):
                so = slice(2 * e, 2 * (e + 1))
                nc.sync.dma_start(qb0[:, so, :], full[("q", 0)][:, so, :])
                nc.scalar.dma_start(wq_sb[:, so, :], wq_r[:, so, :])
            for qq in range(4):
                so = slice(4 * qq, 4 * (qq + 1))
                nc.sync.dma_start(kb0[:, so, :], full[("k", 0)][:, so, :])
                nc.scalar.dma_start(wk_sb[:, so, :], wk_r[:, so, :])
            nc.sync.dma_start(vb0, full[("v", 0)])
            nc.scalar.dma_start(wv_sb, wv_r)
            nc.sync.dma_start(wo_sb, wo_r)
            cur = (qb0, kb0, vb0)

            for j in range(NQ):
                sl = slice(512 * j, 512 * (j + 1))
                qb, kb, vb = cur

                # ---- project chunk j ----
                for h in range(n_heads):
                    ps = pp.tile([P, 512], F32, tag="pp", name=f"psq{j}_{h}")
                    for o in range(SD):
                        nc.tensor.matmul(
                            ps, wq_sb[:, o, DH * h : DH * (h + 1)], qb[:, o, :],
                            start=(o == 0), stop=(o == SD - 1),
                        )
                    nc.vector.tensor_copy(xqT[h][:, sl], ps)

                for h in range(n_heads):
                    ps = pp.tile([P, 512], F32, tag="pp", name=f"psk{j}_{h}")
                    for o in range(SD):
                        nc.tensor.matmul(
                            ps, wk_sb[:, o, DH * h : DH * (h + 1)], kb[:, o, :],
                            start=(o == 0), stop=(o == SD - 1),
                        )
                    nc.vector.tensor_copy(xkT[h][:, sl], ps)

                for st in range(4):
                    ps = pp.tile([P, HD], F32, tag="pp", name=f"psv{j}_{st}")
                    for o in range(SD):
                        nc.tensor.matmul(
                            ps, vb[:, o, P * st : P * (st + 1)], wv_sb[:, o, :],
                            start=(o == 0), stop=(o == SD - 1),
                        )
                    nc.vector.tensor_copy(xv[:, 4 * j + st, :], ps)

                # partial-output staging buffer for this chunk
                pj = pjpool.tile([512, D], BF16, tag="pj", name=f"pj{j}")
                pj_ap[j] = pj
                pj_r[j] = pj.rearrange("(t p) d -> p t d", p=P)

                cur_next = [None, None, None]

                def on_head(h, j=j):
                    if h == 1 and j >= 1:
                        ojs[j - 1] = _rs(nc, pj_ap[j - 1], ojpool, f"oj{j-1}")
                    if h >= 1 and j + 1 < NQ:
                        cur_next[h - 1] = prefetch(j + 1, h - 1)

                if j < NQ - 1:
                    attn_heads(j, 0, 512, on_head)
                    fin_q.extend((ti, dc) for ti in range(4 * j, 4 * (j + 1))
                                 for dc in range(ND))
                    cur = tuple(cur_next)
                else:
                    # last chunk: two half-attentions so its ReduceScatter
                    # splits into an early (hidden) half and a small tail
                    attn_heads(j, 0, 256, on_head)
                    for ti in (4 * j, 4 * j + 1):
                        for dc in range(ND):
                            final_block(ti, dc)
                    oj3a = _rs(nc, pj_ap[j][0:256, :], ojpool, "oj3a")
                    attn_heads(j, 256, 256)
                    for ti in (4 * j + 2, 4 * j + 3):
                        for dc in range(ND):
                            final_block(ti, dc)
                    oj3b = _rs(nc, pj_ap[j][256:512, :], ojpool, "oj3b")

            for jj in range(NQ - 1):
                nc.sync.dma_start(outs[P * jj : P * (jj + 1), :], ojs[jj])
            nc.sync.dma_start(outs[384:448, :], oj3a)
            nc.sync.dma_start(outs[448:512, :], oj3b)

    nc.compile()
    return nc


def _rs(nc, pj, ojpool, name):
    """ReduceScatter a [R, D] partial across the 4-core group; this core
    keeps rows [R//4*r : R//4*(r+1)] (r = its rank)."""
    rows = pj.shape[0]
    oj = ojpool.tile([rows // 4, pj.shape[1]], BF16, tag="oj", name=name)
    nc.gpsimd.collective_compute(
        "ReduceScatter", mybir.AluOpType.add,
        replica_groups=GROUPS, ins=[pj.opt()], outs=[oj.opt()],
    )
    return oj


def make_cmask():
    """cmask[sk_local, sq_local] = 1 if sk_local <= sq_local (bf16)."""
    return np.triu(np.ones((P, P), np.float32)).astype(ml_dtypes.bfloat16)


def run(q, k, v, wq, wk, wv, wo, trace=False, trace_cores=None, **build_kw):
    B, S, D = q.shape
    n_groups = 4  # head groups; 8 cores = B x n_groups
    HD = D // n_groups
    nc = build_nc(S=S, D=D, **build_kw)
    bf = ml_dtypes.bfloat16

    cmask = make_cmask()
    qT = [np.ascontiguousarray(q[b].T).astype(bf) for b in range(B)]
    kT = [np.ascontiguousarray(k[b].T).astype(bf) for b in range(B)]
    vT = [np.ascontiguousarray(v[b].T).astype(bf) for b in range(B)]

    in_maps = []
    for core in range(8):
        b, g = divmod(core, n_groups)
        gs = slice(HD * g, HD * (g + 1))
        m = {
            "qT": qT[b], "kT": kT[b], "vT": vT[b],
            "wq": np.ascontiguousarray(wq[:, gs]).astype(bf),
            "wk": np.ascontiguousarray(wk[:, gs]).astype(bf),
            "wv": np.ascontiguousarray(wv[:, gs]).astype(bf),
            "wo": np.ascontiguousarray(wo[gs, :]).astype(bf),
            "cmask": cmask,
        }
        in_maps.append(m)

    res = run_bass_kernel_spmd(
        nc,
        in_maps,
        core_ids=list(range(8)),
        trace=trace,
        **({"trace_cores": trace_cores} if trace_cores else {}),
    )

    full = np.empty((B, S, D), np.float32)
    for core in range(8):
        b, r = divmod(core, n_groups)
        o = res.results[core]["outs"].astype(np.float32)
        for j in range(S // 512 - 1):
            full[b, 512 * j + P * r : 512 * j + P * (r + 1)] = \
                o[P * j : P * (j + 1)]
        # last chunk arrives as two [256,D] ReduceScatters (64 rows each)
        full[b, 1536 + 64 * r : 1536 + 64 * (r + 1)] = o[384:448]
        full[b, 1792 + 64 * r : 1792 + 64 * (r + 1)] = o[448:512]
    return full, res


def kernel(q, k, v, wq, wk, wv, wo):
    full, _ = run(q, k, v, wq, wk, wv, wo)
    return full


# revision 24
# speedup vs baseline: 1.9552x; 1.0047x over previous
"""Causal multi-head attention (B=2, S=2048, D=2048, H=16, Dh=128) on 8 NeuronCores.

Sharding: 8 cores = 2 batches x 4 head-groups; replica groups
[[0,1,2,3],[4,5,6,7]] (one group per batch element). Core (b,g):
  - receives the FULL transposed activations qT/kT/vT of its batch and its
    head-group's weight slices in local DRAM (host-side replication is free:
    the graded metric is NEFF execution time),
  - projects q,k,v against its 512-column slice of wq/wk/wv,
  - runs causal attention for its 4 heads,
  - multiplies by its 512-row slice of wo -> partial [S, D] output,
  - the partial outputs are summed across the 4-core group with
    ReduceScatters (one per 512-query chunk; the last chunk in two halves
    so the exposed tail is one half-sized collective), each core keeping a
    disjoint row slice.
Host only reorders rows (no arithmetic beyond dtype cast).

Everything bf16 on the wire and in SBUF; PSUM accumulates fp32.

Layout/scheduling notes:
  - Single j-loop over 512-wide query chunks: project chunk j -> attention
    for chunk j (with the PREVIOUS chunk's wo-projection blocks interleaved
    between heads) -> stage chunk j's partial output. Keeps independent PE
    work available so the PE never idles (p-state stays high).
  - Chunk j+1's q/k/v DMAs are issued at head 1 of chunk j's attention, so
    projections never wait on HBM.
  - Scores are computed transposed (scoresT[sk, sq]); softmax denominator
    accumulated on the vector engine (exp-tile running sum), reduced across
    partitions on gpsimd, inverted with the fast DVE reciprocal.
  - Causal handling at 128 granularity: for a tile straddling the diagonal,
    columns left of the tile are skipped and the single 128x128 straddling
    block is masked.
  - score->exp->PV chain pipelined four k-tiles deep.
  - PSUM->SBUF copies of the output projection go through the scalar
    engine (Copy activation) to keep the vector engine off the critical
    path.
"""

import math

import ml_dtypes
import numpy as np

import concourse.bass as bass
import concourse.tile as tile
from concourse import bacc, bass_isa, mybir
from concourse.bass_utils import run_bass_kernel_spmd

F32 = mybir.dt.float32
BF16 = mybir.dt.bfloat16

N_HEADS_PER_CORE = 4
DH = 128
P = 128
GROUPS = [[0, 1, 2, 3], [4, 5, 6, 7]]   # per-batch head-group quartets


def build_nc(S=2048, D=2048, n_heads=N_HEADS_PER_CORE, pt_ahead=4):
    """Build the per-core Bass program. Every core runs this same NEFF."""
    HD = n_heads * DH  # head-group width (columns of wq/wk/wv, rows of wo)
    SD = D // P        # contraction chunks for the projections
    NQ = S // 512      # 512-wide sequence chunks
    NT = S // P        # 128-row sequence tiles
    ND = D // 512      # 512-wide model-dim chunks of the output

    inv_sqrt_dh = 1.0 / math.sqrt(DH)

    nc = bacc.Bacc("TRN2", target_bir_lowering=False, debug=False)

    qT = nc.dram_tensor("qT", [D, S], BF16, kind="ExternalInput").ap()
    kT = nc.dram_tensor("kT", [D, S], BF16, kind="ExternalInput").ap()
    vT = nc.dram_tensor("vT", [D, S], BF16, kind="ExternalInput").ap()
    wq = nc.dram_tensor("wq", [D, HD], BF16, kind="ExternalInput").ap()
    wk = nc.dram_tensor("wk", [D, HD], BF16, kind="ExternalInput").ap()
    wv = nc.dram_tensor("wv", [D, HD], BF16, kind="ExternalInput").ap()
    wo = nc.dram_tensor("wo", [HD, D], BF16, kind="ExternalInput").ap()
    outs = nc.dram_tensor("outs", [512, D], BF16, kind="ExternalOutput").ap()
    cmask = nc.dram_tensor("cmask", [P, P], BF16, kind="ExternalInput").ap()

    wq_r = wq.rearrange("(o p) f -> p o f", p=P)
    wk_r = wk.rearrange("(o p) f -> p o f", p=P)
    wv_r = wv.rearrange("(o p) f -> p o f", p=P)
    wo_r = wo.rearrange("(h p) f -> p h f", p=P)

    with tile.TileContext(nc) as tc:
        with (
            tc.tile_pool(name="consts", bufs=1) as consts,
            tc.tile_pool(name="wpool", bufs=1) as wpool,
            tc.tile_pool(name="bigs", bufs=1) as bigs,
            tc.tile_pool(name="stream", bufs=3) as stream,
            tc.tile_pool(name="ptpool", bufs=6) as ptpool,
            tc.tile_pool(name="dspool", bufs=2) as dspool,
            tc.tile_pool(name="dbpool", bufs=2) as dbpool,
            tc.tile_pool(name="ostage", bufs=10) as ostage,
            tc.tile_pool(name="pp", bufs=2, space="PSUM") as pp,
            tc.tile_pool(name="scp", bufs=4, space="PSUM") as scp,
            tc.tile_pool(name="pvp", bufs=2, space="PSUM") as pvp,
            tc.tile_pool(name="pjp", bufs=4, space="DRAM") as pjpool,
            tc.tile_pool(name="ojp", bufs=6, space="DRAM") as ojpool,
        ):
            cm = consts.tile([P, P], BF16)

            wq_sb = wpool.tile([P, SD, HD], BF16, name="wq_sb")
            wk_sb = wpool.tile([P, SD, HD], BF16, name="wk_sb")
            wv_sb = wpool.tile([P, SD, HD], BF16, name="wv_sb")
            wo_sb = wpool.tile([P, n_heads, D], BF16, name="wo_sb")

            full = {}
            for j in range(NQ):
                sj = slice(512 * j, 512 * (j + 1))
                full[("q", j)] = qT.rearrange("(o p) s -> p o s", p=P)[:, :, sj]
                full[("k", j)] = kT.rearrange("(o p) s -> p o s", p=P)[:, :, sj]
                full[("v", j)] = vT.rearrange("(o p) s -> p o s", p=P)[:, :, sj]

            # persistent activations (feature-major, per head)
            xqT = [bigs.tile([P, S], BF16, name=f"xqT{h}") for h in range(n_heads)]
            xkT = [bigs.tile([P, S], BF16, name=f"xkT{h}") for h in range(n_heads)]
            xv = bigs.tile([P, NT, HD], BF16, name="xv")
            oT = [bigs.tile([P, S], BF16, name=f"oT{h}") for h in range(n_heads)]

            def final_block(ti, dc):
                """One [128sq, 512dc] tile of (sum_h oT_h^T @ wo_h) for chunk
                ti//4, staged to the chunk's partial-output DRAM buffer."""
                fp = pp.tile([P, 512], F32, tag="pp", name=f"fp{ti}_{dc}")
                for h in range(n_heads):
                    nc.tensor.matmul(
                        fp,
                        oT[h][:, P * ti : P * (ti + 1)],
                        wo_sb[:, h, 512 * dc : 512 * (dc + 1)],
                        start=(h == 0), stop=(h == n_heads - 1),
                    )
                stg = ostage.tile([P, 512], BF16, tag="ostage")
                nc.scalar.activation(stg, fp, mybir.ActivationFunctionType.Copy)
                jj = ti // 4
                dst = pj_r[jj][:, ti - 4 * jj, 512 * dc : 512 * (dc + 1)]
                nc.sync.dma_start(dst, stg)

            def prefetch(jn, part):
                """Issue one of chunk jn's activation DMAs in 512KB quarters
                on the scalar queue (own DMA rings -- keeps the staging
                writes on the sync rings from queueing behind 2MB bursts)."""
                t = stream.tile([P, SD, 512], BF16, tag="blk",
                                name="qkv"[part] + "b")
                for qq in range(4):
                    so = slice(4 * qq, 4 * (qq + 1))
                    nc.scalar.dma_start(t[:, so, :],
                                        full[("qkv"[part], jn)][:, so, :])
                return t

            pj_r = {}   # chunk j -> rearranged partial-output DRAM AP
            pj_ap = {}
            ojs = {}    # chunk j -> ReduceScatter output tile
            fin_q = []  # (ti, dc) final blocks not yet emitted
            cur = {}    # chunk j's streamed qb/kb/vb

            def attn_heads(j, q0, qw, on_head=None):
                """Causal attention for queries [512j+q0, 512j+q0+qw), all
                heads, interleaving queued wo-projection blocks."""
                Q0 = 512 * j + q0
                sl = slice(Q0, Q0 + qw)
                nkt = (Q0 + qw) // P
                for h in range(n_heads):
                    pv = pvp.tile([P, qw], F32, tag="pv", name=f"pv{j}_{q0}_{h}")
                    # exp-tile running sum (fp32, vector engine) -- keeps the
                    # softmax denominator off the PE entirely
                    ptsum = dspool.tile([P, qw], F32, tag="ds",
                                        name=f"ds{j}_{q0}_{h}")

                    def make_pt(t, h=h, ptsum=ptsum):
                        off = P * t - Q0
                        c0 = max(0, off)
                        sc = scp.tile([P, qw], F32, tag="sc",
                                      name=f"sc{j}_{q0}_{h}_{t}")
                        nc.tensor.matmul(
                            sc[:, c0:],
                            xkT[h][:, P * t : P * (t + 1)],
                            xqT[h][:, Q0 + c0 : Q0 + qw],
                            start=True, stop=True,
                        )
                        pt = ptpool.tile([P, qw], BF16, tag="pt",
                                         name=f"pt{j}_{q0}_{h}_{t}")
                        nc.scalar.activation(
                            pt[:, c0:], sc[:, c0:],
                            mybir.ActivationFunctionType.Exp, scale=inv_sqrt_dh,
                        )
                        if off >= 0:  # mask the block straddling the diagonal
                            nc.vector.tensor_mul(
                                pt[:, c0 : c0 + P], pt[:, c0 : c0 + P], cm
                            )
                        if t == 0:
                            nc.vector.tensor_copy(ptsum, pt)
                        else:
                            nc.vector.tensor_add(
                                ptsum[:, c0:], ptsum[:, c0:], pt[:, c0:]
                            )
                        return pt, c0

                    pts = [make_pt(tt) for tt in range(min(pt_ahead, nkt))]
                    for t in range(nkt):
                        pt, c0 = pts[t]
                        if t + pt_ahead < nkt:
                            pts.append(make_pt(t + pt_ahead))
                        nc.tensor.matmul(
                            pv[:, c0:],
                            xv[:, t, DH * h : DH * (h + 1)],
                            pt[:, c0:],
                            start=(t == 0), stop=(t == nkt - 1),
                        )

                    # denominator: all-partition sum of ptsum broadcast to
                    # every partition (gpsimd), fast 1/x, then the scale-mul
                    db = dbpool.tile([P, qw], F32, tag="db")
                    nc.gpsimd.partition_all_reduce(
                        db, ptsum, channels=P, reduce_op=bass_isa.ReduceOp.add
                    )
                    dbi = dbpool.tile([P, qw], F32, tag="db")
                    nc.vector.reciprocal_approx_fast(dbi, db)
                    nc.vector.tensor_mul(oT[h][:, sl], pv, dbi)

                    # interleave the previous chunk's output projection
                    # between attention heads to fill exp-latency bubbles
                    for _ in range(8):
                        if fin_q:
                            final_block(*fin_q.pop(0))
                    if on_head is not None:
                        on_head(h)

            # ---- initial loads: chunk 0 interleaved with weights so the
            # first projection matmul starts after ~0.5MB of DMA ----
            nc.scalar.dma_start(cm, cmask)
            qb0 = stream.tile([P, SD, 512], BF16, tag="blk", name="qb")
            kb0 = stream.tile([P, SD, 512], BF16, tag="blk", name="kb")
            vb0 = stream.tile([P, SD, 512], BF16, tag="blk", name="vb")
            for e in range(8):
                so = slice(2 * e, 2 * (e + 1))
                nc.sync.dma_start(qb0[:, so, :], full[("q", 0)][:, so, :])
                nc.scalar.dma_start(wq_sb[:, so, :], wq_r[:, so, :])
            for qq in range(4):
                so = slice(4 * qq, 4 * (qq + 1))
                nc.sync.dma_start(kb0[:, so, :], full[("k", 0)][:, so, :])
                nc.scalar.dma_start(wk_sb[:, so, :], wk_r[:, so, :])
            nc.sync.dma_start(vb0, full[("v", 0)])
            nc.scalar.dma_start(wv_sb, wv_r)
            nc.sync.dma_start(wo_sb, wo_r)
            cur = (qb0, kb0, vb0)

            for j in range(NQ):
                sl = slice(512 * j, 512 * (j + 1))
                qb, kb, vb = cur

                # ---- project chunk j ----
                for h in range(n_heads):
                    ps = pp.tile([P, 512], F32, tag="pp", name=f"psq{j}_{h}")
                    for o in range(SD):
                        nc.tensor.matmul(
                            ps, wq_sb[:, o, DH * h : DH * (h + 1)], qb[:, o, :],
                            start=(o == 0), stop=(o == SD - 1),
                        )
                    nc.vector.tensor_copy(xqT[h][:, sl], ps)

                for h in range(n_heads):
                    ps = pp.tile([P, 512], F32, tag="pp", name=f"psk{j}_{h}")
                    for o in range(SD):
                        nc.tensor.matmul(
                            ps, wk_sb[:, o, DH * h : DH * (h + 1)], kb[:, o, :],
                            start=(o == 0), stop=(o == SD - 1),
                        )
                    nc.vector.tensor_copy(xkT[h][:, sl], ps)

                for st in range(4):
                    ps = pp.tile([P, HD], F32, tag="pp", name=f"psv{j}_{st}")
                    for o in range(SD):
                        nc.tensor.matmul(
                            ps, vb[:, o, P * st : P * (st + 1)], wv_sb[:, o, :],
                            start=(o == 0), stop=(o == SD - 1),
                        )
                    nc.vector.tensor_copy(xv[:, 4 * j + st, :], ps)

                # partial-output staging buffer for this chunk
                pj = pjpool.tile([512, D], BF16, tag="pj", name=f"pj{j}")
                pj_ap[j] = pj
                pj_r[j] = pj.rearrange("(t p) d -> p t d", p=P)

                cur_next = [None, None, None]

                def on_head(h, j=j):
                    if h == 1 and j >= 1:
                        ojs[j - 1] = _rs(nc, pj_ap[j - 1], ojpool, f"oj{j-1}")
                    if h >= 1 and j + 1 < NQ:
                        cur_next[h - 1] = prefetch(j + 1, h - 1)

                if j < NQ - 1:
                    attn_heads(j, 0, 512, on_head)
                    fin_q.extend((ti, dc) for ti in range(4 * j, 4 * (j + 1))
                                 for dc in range(ND))
                    cur = tuple(cur_next)
                else:
                    # last chunk: two half-attentions so its ReduceScatter
                    # splits into an early (hidden) half and a small tail
                    attn_heads(j, 0, 256, on_head)
                    for ti in (4 * j, 4 * j + 1):
                        for dc in range(ND):
                            final_block(ti, dc)
                    oj3a = _rs(nc, pj_ap[j][0:256, :], ojpool, "oj3a")
                    attn_heads(j, 256, 256)
                    for ti in (4 * j + 2, 4 * j + 3):
                        for dc in range(ND):
                            final_block(ti, dc)
                    oj3b = _rs(nc, pj_ap[j][256:512, :], ojpool, "oj3b")

            for jj in range(NQ - 1):
                nc.sync.dma_start(outs[P * jj : P * (jj + 1), :], ojs[jj])
            nc.sync.dma_start(outs[384:448, :], oj3a)
            nc.sync.dma_start(outs[448:512, :], oj3b)

    nc.compile()
    return nc


def _rs(nc, pj, ojpool, name):
    """ReduceScatter a [R, D] partial across the 4-core group; this core
    keeps rows [R//4*r : R//4*(r+1)] (r = its rank)."""
    rows = pj.shape[0]
    oj = ojpool.tile([rows // 4, pj.shape[1]], BF16, tag="oj", name=name)
    nc.gpsimd.collective_compute(
        "ReduceScatter", mybir.AluOpType.add,
        replica_groups=GROUPS, ins=[pj.opt()], outs=[oj.opt()],
    )
    return oj


def make_cmask():
    """cmask[sk_local, sq_local] = 1 if sk_local <= sq_local (bf16)."""
    return np.triu(np.ones((P, P), np.float32)).astype(ml_dtypes.bfloat16)


def run(q, k, v, wq, wk, wv, wo, trace=False, trace_cores=None, **build_kw):
    B, S, D = q.shape
    n_groups = 4  # head groups; 8 cores = B x n_groups
    HD = D // n_groups
    nc = build_nc(S=S, D=D, **build_kw)
    bf = ml_dtypes.bfloat16

    cmask = make_cmask()
    qT = [np.ascontiguousarray(q[b].T).astype(bf) for b in range(B)]
    kT = [np.ascontiguousarray(k[b].T).astype(bf) for b in range(B)]
    vT = [np.ascontiguousarray(v[b].T).astype(bf) for b in range(B)]

    in_maps = []
    for core in range(8):
        b, g = divmod(core, n_groups)
        gs = slice(HD * g, HD * (g + 1))
        m = {
            "qT": qT[b], "kT": kT[b], "vT": vT[b],
            "wq": np.ascontiguousarray(wq[:, gs]).astype(bf),
            "wk": np.ascontiguousarray(wk[:, gs]).astype(bf),
            "wv": np.ascontiguousarray(wv[:, gs]).astype(bf),
            "wo": np.ascontiguousarray(wo[gs, :]).astype(bf),
            "cmask": cmask,
        }
        in_maps.append(m)

    res = run_bass_kernel_spmd(
        nc,
        in_maps,
        core_ids=list(range(8)),
        trace=trace,
        **({"trace_cores": trace_cores} if trace_cores else {}),
    )

    full = np.empty((B, S, D), np.float32)
    for core in range(8):
        b, r = divmod(core, n_groups)
        o = res.results[core]["outs"].astype(np.float32)
        for j in range(S // 512 - 1):
            full[b, 512 * j + P * r : 512 * j + P * (r + 1)] = \
                o[P * j : P * (j + 1)]
        # last chunk arrives as two [256,D] ReduceScatters (64 rows each)
        full[b, 1536 + 64 * r : 1536 + 64 * (r + 1)] = o[384:448]
        full[b, 1792 + 64 * r : 1792 + 64 * (r + 1)] = o[448:512]
    return full, res


def kernel(q, k, v, wq, wk, wv, wo):
    full, _ = run(q, k, v, wq, wk, wv, wo)
    return full


# revision 25
# speedup vs baseline: 2.0016x; 1.0237x over previous
"""Causal multi-head attention (B=2, S=2048, D=2048, H=16, Dh=128) on 8 NeuronCores.

Sharding: 8 cores = 2 batches x 4 head-groups; replica groups
[[0,1,2,3],[4,5,6,7]] (one group per batch element). Core (b,g):
  - receives the FULL transposed activations qT/kT/vT of its batch and its
    head-group's weight slices in local DRAM (host-side replication is free:
    the graded metric is NEFF execution time),
  - projects q,k,v against its 512-column slice of wq/wk/wv,
  - runs causal attention for its 4 heads,
  - multiplies by its 512-row slice of wo -> partial [S, D] output,
  - the partial outputs are summed across the 4-core group with
    ReduceScatters (one per 512-query chunk; the last chunk in two halves
    so the exposed tail is one half-sized collective), each core keeping a
    disjoint row slice.
Host only reorders rows (no arithmetic beyond dtype cast).

Everything bf16 on the wire and in SBUF; PSUM accumulates fp32.

Layout/scheduling notes:
  - Single j-loop over 512-wide query chunks: project chunk j -> attention
    for chunk j (with the PREVIOUS chunk's wo-projection blocks interleaved
    between heads) -> stage chunk j's partial output. Keeps independent PE
    work available so the PE never idles (p-state stays high).
  - Chunk j+1's q/k/v DMAs are issued at head 1 of chunk j's attention, so
    projections never wait on HBM.
  - Scores are computed transposed (scoresT[sk, sq]); softmax denominator
    accumulated on the vector engine (exp-tile running sum), reduced across
    partitions on gpsimd, inverted with the fast DVE reciprocal.
  - Causal handling at 128 granularity: for a tile straddling the diagonal,
    columns left of the tile are skipped and the single 128x128 straddling
    block is masked.
  - score->exp->PV chain pipelined four k-tiles deep.
  - PSUM->SBUF copies of the output projection go through the scalar
    engine (Copy activation) to keep the vector engine off the critical
    path.
"""

import math

import ml_dtypes
import numpy as np

import concourse.bass as bass
import concourse.tile as tile
from concourse import bacc, bass_isa, mybir
from concourse.bass_utils import run_bass_kernel_spmd

F32 = mybir.dt.float32
BF16 = mybir.dt.bfloat16

N_HEADS_PER_CORE = 4
DH = 128
P = 128
GROUPS = [[0, 1, 2, 3], [4, 5, 6, 7]]   # per-batch head-group quartets


def build_nc(S=2048, D=2048, n_heads=N_HEADS_PER_CORE, pt_ahead=4):
    """Build the per-core Bass program. Every core runs this same NEFF."""
    HD = n_heads * DH  # head-group width (columns of wq/wk/wv, rows of wo)
    SD = D // P        # contraction chunks for the projections
    NQ = S // 512      # 512-wide sequence chunks
    NT = S // P        # 128-row sequence tiles
    ND = D // 512      # 512-wide model-dim chunks of the output

    inv_sqrt_dh = 1.0 / math.sqrt(DH)

    nc = bacc.Bacc("TRN2", target_bir_lowering=False, debug=False)

    qT = nc.dram_tensor("qT", [D, S], BF16, kind="ExternalInput").ap()
    kT = nc.dram_tensor("kT", [D, S], BF16, kind="ExternalInput").ap()
    vT = nc.dram_tensor("vT", [D, S], BF16, kind="ExternalInput").ap()
    wq = nc.dram_tensor("wq", [D, HD], BF16, kind="ExternalInput").ap()
    wk = nc.dram_tensor("wk", [D, HD], BF16, kind="ExternalInput").ap()
    wv = nc.dram_tensor("wv", [D, HD], BF16, kind="ExternalInput").ap()
    wo = nc.dram_tensor("wo", [HD, D], BF16, kind="ExternalInput").ap()
    outs = nc.dram_tensor("outs", [512, D], BF16, kind="ExternalOutput").ap()
    cmask = nc.dram_tensor("cmask", [P, P], BF16, kind="ExternalInput").ap()

    wq_r = wq.rearrange("(o p) f -> p o f", p=P)
    wk_r = wk.rearrange("(o p) f -> p o f", p=P)
    wv_r = wv.rearrange("(o p) f -> p o f", p=P)
    wo_r = wo.rearrange("(h p) f -> p h f", p=P)

    with tile.TileContext(nc) as tc:
        with (
            tc.tile_pool(name="consts", bufs=1) as consts,
            tc.tile_pool(name="wpool", bufs=1) as wpool,
            tc.tile_pool(name="bigs", bufs=1) as bigs,
            tc.tile_pool(name="stream", bufs=3) as stream,
            tc.tile_pool(name="ptpool", bufs=6) as ptpool,
            tc.tile_pool(name="dspool", bufs=2) as dspool,
            tc.tile_pool(name="dbpool", bufs=2) as dbpool,
            tc.tile_pool(name="ostage", bufs=10) as ostage,
            tc.tile_pool(name="pp", bufs=2, space="PSUM") as pp,
            tc.tile_pool(name="scp", bufs=4, space="PSUM") as scp,
            tc.tile_pool(name="pvp", bufs=2, space="PSUM") as pvp,
            tc.tile_pool(name="pjp", bufs=4, space="DRAM") as pjpool,
            tc.tile_pool(name="ojp", bufs=6, space="DRAM") as ojpool,
        ):
            cm = consts.tile([P, P], BF16)

            wq_sb = wpool.tile([P, SD, HD], BF16, name="wq_sb")
            wk_sb = wpool.tile([P, SD, HD], BF16, name="wk_sb")
            wv_sb = wpool.tile([P, SD, HD], BF16, name="wv_sb")
            wo_sb = wpool.tile([P, n_heads, D], BF16, name="wo_sb")

            full = {}
            for j in range(NQ):
                sj = slice(512 * j, 512 * (j + 1))
                full[("q", j)] = qT.rearrange("(o p) s -> p o s", p=P)[:, :, sj]
                full[("k", j)] = kT.rearrange("(o p) s -> p o s", p=P)[:, :, sj]
                full[("v", j)] = vT.rearrange("(o p) s -> p o s", p=P)[:, :, sj]

            # persistent activations (feature-major, per head)
            xqT = [bigs.tile([P, S], BF16, name=f"xqT{h}") for h in range(n_heads)]
            xkT = [bigs.tile([P, S], BF16, name=f"xkT{h}") for h in range(n_heads)]
            xv = bigs.tile([P, NT, HD], BF16, name="xv")
            oT = [bigs.tile([P, S], BF16, name=f"oT{h}") for h in range(n_heads)]

            def final_block(ti, dc):
                """One [128sq, 512dc] tile of (sum_h oT_h^T @ wo_h) for chunk
                ti//4, staged to the chunk's partial-output DRAM buffer."""
                fp = pp.tile([P, 512], F32, tag="pp", name=f"fp{ti}_{dc}")
                for h in range(n_heads):
                    nc.tensor.matmul(
                        fp,
                        oT[h][:, P * ti : P * (ti + 1)],
                        wo_sb[:, h, 512 * dc : 512 * (dc + 1)],
                        start=(h == 0), stop=(h == n_heads - 1),
                    )
                stg = ostage.tile([P, 512], BF16, tag="ostage")
                nc.scalar.activation(stg, fp, mybir.ActivationFunctionType.Copy)
                jj = ti // 4
                dst = pj_r[jj][:, ti - 4 * jj, 512 * dc : 512 * (dc + 1)]
                nc.sync.dma_start(dst, stg)

            def prefetch(jn, part):
                """Issue one of chunk jn's activation DMAs in 512KB quarters
                on the scalar queue (own DMA rings -- keeps the staging
                writes on the sync rings from queueing behind 2MB bursts)."""
                t = stream.tile([P, SD, 512], BF16, tag="blk",
                                name="qkv"[part] + "b")
                for qq in range(4):
                    so = slice(4 * qq, 4 * (qq + 1))
                    nc.scalar.dma_start(t[:, so, :],
                                        full[("qkv"[part], jn)][:, so, :])
                return t

            pj_r = {}   # chunk j -> rearranged partial-output DRAM AP
            pj_ap = {}
            ojs = {}    # chunk j -> ReduceScatter output tile
            fin_q = []  # (ti, dc) final blocks not yet emitted
            cur = {}    # chunk j's streamed qb/kb/vb

            def attn_heads(j, q0, qw, on_head=None):
                """Causal attention for queries [512j+q0, 512j+q0+qw), all
                heads, interleaving queued wo-projection blocks."""
                Q0 = 512 * j + q0
                sl = slice(Q0, Q0 + qw)
                nkt = (Q0 + qw) // P
                for h in range(n_heads):
                    pv = pvp.tile([P, qw], F32, tag="pv", name=f"pv{j}_{q0}_{h}")
                    # exp-tile running sum (fp32, vector engine) -- keeps the
                    # softmax denominator off the PE entirely
                    ptsum = dspool.tile([P, qw], F32, tag="ds",
                                        name=f"ds{j}_{q0}_{h}")

                    def make_pt(t, h=h, ptsum=ptsum):
                        off = P * t - Q0
                        c0 = max(0, off)
                        sc = scp.tile([P, qw], F32, tag="sc",
                                      name=f"sc{j}_{q0}_{h}_{t}")
                        nc.tensor.matmul(
                            sc[:, c0:],
                            xkT[h][:, P * t : P * (t + 1)],
                            xqT[h][:, Q0 + c0 : Q0 + qw],
                            start=True, stop=True,
                        )
                        pt = ptpool.tile([P, qw], BF16, tag="pt",
                                         name=f"pt{j}_{q0}_{h}_{t}")
                        nc.scalar.activation(
                            pt[:, c0:], sc[:, c0:],
                            mybir.ActivationFunctionType.Exp, scale=inv_sqrt_dh,
                        )
                        if off >= 0:  # mask the block straddling the diagonal
                            nc.vector.tensor_mul(
                                pt[:, c0 : c0 + P], pt[:, c0 : c0 + P], cm
                            )
                        if t == 0:
                            nc.vector.tensor_copy(ptsum, pt)
                        else:
                            nc.vector.tensor_add(
                                ptsum[:, c0:], ptsum[:, c0:], pt[:, c0:]
                            )
                        return pt, c0

                    pts = [make_pt(tt) for tt in range(min(pt_ahead, nkt))]
                    for t in range(nkt):
                        pt, c0 = pts[t]
                        if t + pt_ahead < nkt:
                            pts.append(make_pt(t + pt_ahead))
                        nc.tensor.matmul(
                            pv[:, c0:],
                            xv[:, t, DH * h : DH * (h + 1)],
                            pt[:, c0:],
                            start=(t == 0), stop=(t == nkt - 1),
                        )

                    # denominator: all-partition sum of ptsum broadcast to
                    # every partition (gpsimd), fast 1/x, then the scale-mul
                    db = dbpool.tile([P, qw], F32, tag="db")
                    nc.gpsimd.partition_all_reduce(
                        db, ptsum, channels=P, reduce_op=bass_isa.ReduceOp.add
                    )
                    dbi = dbpool.tile([P, qw], F32, tag="db")
                    nc.vector.reciprocal_approx_fast(dbi, db)
                    nc.vector.tensor_mul(oT[h][:, sl], pv, dbi)

                    # interleave the previous chunk's output projection
                    # between attention heads to fill exp-latency bubbles
                    for _ in range(8):
                        if fin_q:
                            final_block(*fin_q.pop(0))
                    if on_head is not None:
                        on_head(h)

            # ---- initial loads: chunk 0 interleaved with weights so the
            # first projection matmul starts after ~0.5MB of DMA ----
            nc.scalar.dma_start(cm, cmask)
            qb0 = stream.tile([P, SD, 512], BF16, tag="blk", name="qb")
            kb0 = stream.tile([P, SD, 512], BF16, tag="blk", name="kb")
            vb0 = stream.tile([P, SD, 512], BF16, tag="blk", name="vb")
            for e in range(8):
                so = slice(2 * e, 2 * (e + 1))
                nc.sync.dma_start(qb0[:, so, :], full[("q", 0)][:, so, :])
                nc.scalar.dma_start(wq_sb[:, so, :], wq_r[:, so, :])
            for qq in range(4):
                so = slice(4 * qq, 4 * (qq + 1))
                nc.sync.dma_start(kb0[:, so, :], full[("k", 0)][:, so, :])
                nc.scalar.dma_start(wk_sb[:, so, :], wk_r[:, so, :])
            nc.sync.dma_start(vb0, full[("v", 0)])
            nc.scalar.dma_start(wv_sb, wv_r)
            cur = (qb0, kb0, vb0)

            for j in range(NQ):
                sl = slice(512 * j, 512 * (j + 1))
                qb, kb, vb = cur

                # ---- project chunk j ----
                for h in range(n_heads):
                    ps = pp.tile([P, 512], F32, tag="pp", name=f"psq{j}_{h}")
                    for o in range(SD):
                        nc.tensor.matmul(
                            ps, wq_sb[:, o, DH * h : DH * (h + 1)], qb[:, o, :],
                            start=(o == 0), stop=(o == SD - 1),
                        )
                    nc.vector.tensor_copy(xqT[h][:, sl], ps)

                for h in range(n_heads):
                    ps = pp.tile([P, 512], F32, tag="pp", name=f"psk{j}_{h}")
                    for o in range(SD):
                        nc.tensor.matmul(
                            ps, wk_sb[:, o, DH * h : DH * (h + 1)], kb[:, o, :],
                            start=(o == 0), stop=(o == SD - 1),
                        )
                    nc.vector.tensor_copy(xkT[h][:, sl], ps)

                for st in range(4):
                    ps = pp.tile([P, HD], F32, tag="pp", name=f"psv{j}_{st}")
                    for o in range(SD):
                        nc.tensor.matmul(
                            ps, vb[:, o, P * st : P * (st + 1)], wv_sb[:, o, :],
                            start=(o == 0), stop=(o == SD - 1),
                        )
                    nc.vector.tensor_copy(xv[:, 4 * j + st, :], ps)

                # partial-output staging buffer for this chunk
                pj = pjpool.tile([512, D], BF16, tag="pj", name=f"pj{j}")
                pj_ap[j] = pj
                pj_r[j] = pj.rearrange("(t p) d -> p t d", p=P)

                cur_next = [None, None, None]

                def on_head(h, j=j):
                    if h == 0 and j == 0:
                        nc.sync.dma_start(wo_sb, wo_r)
                    if h == 1 and 1 <= j < NQ - 1:
                        ojs[j - 1] = _rs(nc, pj_ap[j - 1], ojpool, f"oj{j-1}")
                    if j == NQ - 1:
                        # split the previous chunk's RS in two so the CC
                        # stream clears before the endgame collectives
                        if h == 0:
                            ojs["2a"] = _rs(nc, pj_ap[j - 1][0:256, :],
                                            ojpool, "oj2a")
                        elif h == 1:
                            ojs["2b"] = _rs(nc, pj_ap[j - 1][256:512, :],
                                            ojpool, "oj2b")
                    if h >= 1 and j + 1 < NQ:
                        cur_next[h - 1] = prefetch(j + 1, h - 1)

                if j < NQ - 1:
                    attn_heads(j, 0, 512, on_head)
                    fin_q.extend((ti, dc) for ti in range(4 * j, 4 * (j + 1))
                                 for dc in range(ND))
                    cur = tuple(cur_next)
                else:
                    # last chunk: two half-attentions so its ReduceScatter
                    # splits into an early (hidden) half and a small tail
                    attn_heads(j, 0, 256, on_head)
                    for ti in (4 * j, 4 * j + 1):
                        for dc in range(ND):
                            final_block(ti, dc)

                    def on_head_b(h, j=j):
                        # trigger RS3a here: its staging-DMA wait must not
                        # block half B's h0/h1 partition_all_reduces on the
                        # gpsimd queue
                        if h == 1:
                            ojs["3a"] = _rs(nc, pj_ap[j][0:256, :],
                                            ojpool, "oj3a")

                    attn_heads(j, 256, 256, on_head_b)
                    for ti in (4 * j + 2, 4 * j + 3):
                        for dc in range(ND):
                            final_block(ti, dc)
                    oj3b = _rs(nc, pj_ap[j][256:512, :], ojpool, "oj3b")

            for jj in range(NQ - 2):
                nc.sync.dma_start(outs[P * jj : P * (jj + 1), :], ojs[jj])
            nc.sync.dma_start(outs[256:320, :], ojs["2a"])
            nc.sync.dma_start(outs[320:384, :], ojs["2b"])
            nc.sync.dma_start(outs[384:448, :], ojs["3a"])
            nc.sync.dma_start(outs[448:512, :], oj3b)

    nc.compile()
    return nc


def _rs(nc, pj, ojpool, name):
    """ReduceScatter a [R, D] partial across the 4-core group; this core
    keeps rows [R//4*r : R//4*(r+1)] (r = its rank)."""
    rows = pj.shape[0]
    oj = ojpool.tile([rows // 4, pj.shape[1]], BF16, tag="oj", name=name)
    nc.gpsimd.collective_compute(
        "ReduceScatter", mybir.AluOpType.add,
        replica_groups=GROUPS, ins=[pj.opt()], outs=[oj.opt()],
    )
    return oj


def make_cmask():
    """cmask[sk_local, sq_local] = 1 if sk_local <= sq_local (bf16)."""
    return np.triu(np.ones((P, P), np.float32)).astype(ml_dtypes.bfloat16)


def run(q, k, v, wq, wk, wv, wo, trace=False, trace_cores=None, **build_kw):
    B, S, D = q.shape
    n_groups = 4  # head groups; 8 cores = B x n_groups
    HD = D // n_groups
    nc = build_nc(S=S, D=D, **build_kw)
    bf = ml_dtypes.bfloat16

    cmask = make_cmask()
    qT = [np.ascontiguousarray(q[b].T).astype(bf) for b in range(B)]
    kT = [np.ascontiguousarray(k[b].T).astype(bf) for b in range(B)]
    vT = [np.ascontiguousarray(v[b].T).astype(bf) for b in range(B)]

    in_maps = []
    for core in range(8):
        b, g = divmod(core, n_groups)
        gs = slice(HD * g, HD * (g + 1))
        m = {
            "qT": qT[b], "kT": kT[b], "vT": vT[b],
            "wq": np.ascontiguousarray(wq[:, gs]).astype(bf),
            "wk": np.ascontiguousarray(wk[:, gs]).astype(bf),
            "wv": np.ascontiguousarray(wv[:, gs]).astype(bf),
            "wo": np.ascontiguousarray(wo[gs, :]).astype(bf),
            "cmask": cmask,
        }
        in_maps.append(m)

    res = run_bass_kernel_spmd(
        nc,
        in_maps,
        core_ids=list(range(8)),
        trace=trace,
        **({"trace_cores": trace_cores} if trace_cores else {}),
    )

    full = np.empty((B, S, D), np.float32)
    for core in range(8):
        b, r = divmod(core, n_groups)
        o = res.results[core]["outs"].astype(np.float32)
        for j in range(S // 512 - 2):
            full[b, 512 * j + P * r : 512 * j + P * (r + 1)] = \
                o[P * j : P * (j + 1)]
        # last two chunks arrive as [256,D] ReduceScatters (64 rows each)
        full[b, 1024 + 64 * r : 1024 + 64 * (r + 1)] = o[256:320]
        full[b, 1280 + 64 * r : 1280 + 64 * (r + 1)] = o[320:384]
        full[b, 1536 + 64 * r : 1536 + 64 * (r + 1)] = o[384:448]
        full[b, 1792 + 64 * r : 1792 + 64 * (r + 1)] = o[448:512]
    return full, res


def kernel(q, k, v, wq, wk, wv, wo):
    full, _ = run(q, k, v, wq, wk, wv, wo)
    return full
